# revision 49
# baseline (speedup 1.0000x reference)
"""Bass/Trainium2 kernel for nn_Attention (B=4, N=2048, IN=256, HID=1024,
D=1024, OUT=256, H=8 heads), SPMD over 8 NeuronCores.

Sharding: core c handles batch b = c//2 and head-group g = c%2 (4 heads,
512 of the 1024 inner features).  Layer-1 of each QKV MLP is recomputed on
both cores of a batch (cheap); the output projection is computed per
head-group and the two partial products are summed on the host (plus bias).

Mask compaction: ~half the tokens are masked out (key mask) and masked
queries only output the bias row.  The host applies ONE permutation
(valid tokens first) to q, k and v inputs, so the kernel runs on
NP = ceil(max_valid/128)*128 tokens instead of N=2048.  Padded key rows
get an additive -30000 before exp (as the per-partition Exp bias).

All matmuls run in bf16.  The query axis is additionally trimmed to
NQ = ceil(max_valid/64)*64 columns (padded queries are discarded on the
host, so nothing reads them).  DMA triggers cost ~0.6us each on their
issuing queue, so inputs are fetched as ONE merged tile per tensor, the
big w1/x tiles stream on the sync queue in first-use order (k's are
split in half so compute starts earlier), w2/constants ride the scalar
queue, and tiny bias tables are packed into one [128,32] tile.  A short
garbage-data matmul warmup ramps the PE p-state (full clock needs ~3us
of continuous execution, and any tensor-queue gap drops it back for
~3us) while the first input tiles stream in; the whole schedule is built
to keep the tensor queue gap-free.

Schedule (the Exp stream on the Scalar engine is the attention limiter,
so three heads' score/exp work runs inside the v-L2 window where Scalar
is otherwise idle):
  1. k-L1; then q-L1 with k-L2 interleaved; then v-L1 with q-L2
     interleaved (keeps tanh/Identity off the critical path)
  2. the 81 v-L2 matmuls interleaved 3-per-S-unit with the 27 S-units of
     heads 0..2
     (S-unit kt: S^T tile [128,NQ] = kT_kt.T @ qT via chunk matmuls;
      Exp with key-mask partition bias -> pt bf16; diagonal zeroed on
      GPSIMD (pt *= 1-I); denominator running-sum on DVE; per head,
      deferred 2 S-units past its last exp: per-chunk all-ones
      stationary matmul -> broadcast sums -> DVE reciprocal_approx_fast
      -> rb[hd])
  3. S-units of head 3 (front-loaded by one slot) interleaved with the
     y2-groups of heads 0..2
     (y2-group (hd,c): 9 accumulating AV matmuls into a 1-bank PSUM
      chunk, then ysc[hd][:,c] = y2c * rb[hd][:,c] on DVE), then
     y2-groups of head 3 with the projection matmuls and bf16 output
     copies/DMAs interleaved right behind them.

PSUM: "big" pool 2 x 3 banks (L1/L2 accumulators and S^T tiles), "small"
pool 2 x 1 bank (warmup, v-L2 tiles, denominator chunks, y2 chunks,
projection).
"""

import numpy as np

B, N, IN_DIM, HID, D, OUT_DIM, H = 4, 2048, 256, 1024, 1024, 256, 8
NCORES = 8
HG = 2                 # head groups (cores per batch)
DG = D // HG           # 512 features per group
HEADS_G = H // HG      # 4 heads per core
Dh = D // H            # 128
NEG = -30000.0         # additive mask value (exp underflows to 0)

_CACHE = {}


def _chunks(total, size):
    out = []
    o = 0
    while o < total:
        s = min(size, total - o)
        out.append((o, s))
        o += s
    return out


def _build_nc(NP, NQ):
    import concourse.mybir as mybir
    import concourse.tile as tile
    from concourse import bacc
    from contextlib import ExitStack

    dt = mybir.dt
    f32 = dt.float32
    bf16 = dt.bfloat16
    AF = mybir.ActivationFunctionType
    ALU = mybir.AluOpType

    # Keep all used activation funcs (Tanh, Exp) in ONE table set so the
    # table-load pass never thrashes.
    if not getattr(bacc, "_act_tables_patched", False):
        from concourse import hw_specs as _hw
        _orig_get = _hw.get_activation_tables

        def _patched(arch):
            tables = dict(_orig_get(arch))
            AFT = mybir.ActivationFunctionType
            keep = {"exp_and_others", "natural_log_exp_and_others"}
            for name in tables:
                if name in keep:
                    continue
                fns = tables[name]
                if AFT.Exp in fns or AFT.Ln in fns:
                    tables[name] = set()
            return tables

        _patched.__wrapped__ = _orig_get
        bacc.get_activation_tables = _patched
        bacc._act_tables_patched = True

    nc = bacc.Bacc("TRN2", target_bir_lowering=False, debug=False)

    # ---- DRAM I/O ----
    xd_ = {}
    w1_ = {}
    w2_ = {}
    for t in ("k", "q", "v"):
        xd_[t] = nc.dram_tensor(f"x{t}T", [IN_DIM, NP], bf16,
                                kind="ExternalInput")
        w1_[t] = nc.dram_tensor(f"w{t}1", [IN_DIM, HID], bf16,
                                kind="ExternalInput")
        w2_[t] = nc.dram_tensor(f"w{t}2", [HID, DG], bf16,
                                kind="ExternalInput")
    bpk = nc.dram_tensor("bpk", [128, 32], f32, kind="ExternalInput")
    bv2row = nc.dram_tensor("bv2row", [128, DG], bf16, kind="ExternalInput")
    e0d = nc.dram_tensor("e0d", [128, 128], bf16, kind="ExternalInput")
    onesd = nc.dram_tensor("onesd", [128, 128], bf16, kind="ExternalInput")
    eyeCd = nc.dram_tensor("eyeCd", [128, 128], bf16, kind="ExternalInput")
    kmd = nc.dram_tensor("kmd", [128, NP // 128], f32, kind="ExternalInput")
    wpb = nc.dram_tensor("wpb", [DG, OUT_DIM], bf16, kind="ExternalInput")
    outT = nc.dram_tensor("outT", [OUT_DIM, NP], bf16, kind="ExternalOutput")

    KT1 = IN_DIM // 128          # 2  k-tiles in layer 1
    KT2 = HID // 128             # 8  k-tiles in layer 2
    MT1 = HID // 128             # 8  m-tiles in layer 1
    NTOK = NP // 128             # key-token tiles
    CK = _chunks(NP, 512)        # key/value token chunks (bank-aligned)
    CQ = _chunks(NQ, 512)        # query token chunks (trimmed to valid)
    NPB = ((NP + 511) // 512) * 512   # psum cols rounded to full banks
    # bias-pack column offsets: b1 per type (8 each), then b2q, b2k (4 each)
    B1OFF = {"v": 0, "k": 8, "q": 16}
    B2OFF = {"q": 24, "k": 28}

    with tile.TileContext(nc) as tc, ExitStack() as ctx:
        # PSUM: big = 2 x 3 banks, small = 2 x 1 bank  (8 banks total)
        big = ctx.enter_context(tc.tile_pool(name="big", bufs=2,
                                             space="PSUM"))
        small = ctx.enter_context(tc.tile_pool(name="small", bufs=2,
                                               space="PSUM"))
        singles = ctx.enter_context(tc.tile_pool(name="singles", bufs=1))
        xt_pool = ctx.enter_context(tc.tile_pool(name="xt", bufs=2))
        w1_pool = ctx.enter_context(tc.tile_pool(name="w1", bufs=2))
        w2_pool = ctx.enter_context(tc.tile_pool(name="w2", bufs=2))
        h_pool = ctx.enter_context(tc.tile_pool(name="h", bufs=12))
        qk_pool = ctx.enter_context(tc.tile_pool(name="qk", bufs=2))
        v_pool = ctx.enter_context(
            tc.tile_pool(name="v", bufs=(NTOK + 3) // 4))
        pt_pool = ctx.enter_context(tc.tile_pool(name="pt", bufs=28))
        sacc_pool = ctx.enter_context(tc.tile_pool(name="sacc", bufs=2))
        rb_pool = ctx.enter_context(tc.tile_pool(name="rb", bufs=3))
        ysc_pool = ctx.enter_context(tc.tile_pool(name="ysc", bufs=4))
        out_pool = ctx.enter_context(tc.tile_pool(name="out", bufs=4))

        # ---- warmup: ramp the PE p-state on zeroed garbage data (the PE
        # needs ~3us of continuous execution to reach full clock; any idle
        # gap drops it back for the next ~3us, so the schedule below is
        # built to keep the tensor queue gap-free) ----
        wu = singles.tile([128, 512], bf16, tag="wu")
        nc.gpsimd.memset(wu[:, :], 0)
        wups = small.tile([128, 512], f32, tag="small")
        for _ in range(8):
            nc.tensor.matmul(wups[:, :], wu[:, :128], wu[:, :],
                             start=True, stop=True)
        nc.vector.tensor_copy(out=wu[:, 0:1], in_=wups[:, 0:1])

        # ---- small constants on the scalar queue (idle at start); w2
        # weights also go there per-type so the sync queue streams only
        # the critical w1/x tiles in first-use order ----
        bpk_sb = singles.tile([128, 32], f32, tag="bpk")
        nc.scalar.dma_start(out=bpk_sb, in_=bpk[:, :])
        ones_sb = singles.tile([128, 128], bf16, tag="ones")
        nc.scalar.dma_start(out=ones_sb, in_=onesd[:, :])
        eyeC_sb = singles.tile([128, 128], bf16, tag="eyeC")
        nc.scalar.dma_start(out=eyeC_sb, in_=eyeCd[:, :])
        km_sb = singles.tile([128, NP // 128], f32, tag="km")
        nc.scalar.dma_start(out=km_sb, in_=kmd[:, :])
        wp_sb = singles.tile([128, HEADS_G, OUT_DIM], bf16, tag="wp")
        nc.scalar.dma_start(
            out=wp_sb, in_=wpb.rearrange("(h p) o -> p h o", p=128))
        bv2_sb = singles.tile([128, DG], bf16, tag="bv2")
        nc.scalar.dma_start(out=bv2_sb, in_=bv2row[:, :])
        e0_sb = singles.tile([128, 128], bf16, tag="e0")
        nc.scalar.dma_start(out=e0_sb, in_=e0d[:, :])

        # persistent activations
        qT = qk_pool.tile([128, HEADS_G, NP], bf16, tag="qk", name="qT")
        kT = qk_pool.tile([128, HEADS_G, NP], bf16, tag="qk", name="kT")
        v_sb = [v_pool.tile([128, 4 * DG], bf16, tag="v", name=f"v{i}")
                for i in range((NTOK + 3) // 4)]

        # ---- S-unit / denominator helpers (used from phase 1 onward) ----
        pts = {}
        rb = {}
        saccs = {}
        sacc_cur = [None]

        def emit_s_unit(hd, kt):
            st = big.tile([128, NPB], f32, tag="big", name="st")
            for c0, cs in CQ:
                nc.tensor.matmul(
                    st[:, c0:c0 + cs],
                    kT[:, hd, kt * 128:(kt + 1) * 128],
                    qT[:, hd, c0:c0 + cs],
                    start=True, stop=True,
                )
            pt = pt_pool.tile([128, NP], bf16, tag="pt", name="pt")
            nc.scalar.activation(
                out=pt[:, :NQ], in_=st[:, :NQ], func=AF.Exp,
                bias=km_sb[:, kt:kt + 1], scale=1.0,
            )
            # no self-attention: zero the diagonal block on GPSIMD
            db = kt * 128
            dw = min(128, NQ - db)
            if dw > 0:
                nc.gpsimd.tensor_tensor(
                    pt[:, db:db + dw], pt[:, db:db + dw], eyeC_sb[:, :dw],
                    ALU.mult)
            if kt == 0:
                sacc_cur[0] = sacc_pool.tile([128, NP], bf16, tag="sacc",
                                             name="sacc")
                nc.vector.tensor_copy(out=sacc_cur[0][:, :NQ],
                                      in_=pt[:, :NQ])
            else:
                nc.vector.tensor_tensor(sacc_cur[0][:, :NQ],
                                        sacc_cur[0][:, :NQ], pt[:, :NQ],
                                        ALU.add)
            pts[(hd, kt)] = pt
            if kt == NTOK - 1:
                saccs[hd] = sacc_cur[0]

        def emit_aux(hd):
            # denominators -> broadcast sums -> 1/s.  Deferred a couple of
            # S-units past the head's last exp so the tensor queue never
            # stalls waiting for the DVE running-sum chain to finish.
            rbt = rb_pool.tile([128, NP], f32, tag="rb", name="rbt")
            for c0, cs in CQ:
                aux = small.tile([128, 512], f32, tag="small", name="aux")
                nc.tensor.matmul(
                    aux[:, :cs], ones_sb[:, :], saccs[hd][:, c0:c0 + cs],
                    start=True, stop=True,
                )
                nc.vector.reciprocal_approx_fast(
                    out=rbt[:, c0:c0 + cs], in_=aux[:, :cs])
            rb[hd] = rbt

        # ---------------- phase 1: k-MLP, q-MLP, v-L1 --------------------
        # Six S-units of head 0 are interleaved into q-L2 (the Scalar
        # engine has tanh-free slack there), shrinking phase 2's exp floor.
        def emit_type_dma(t):
            # x tiles stream on the sync queue, w1 tiles in parallel on the
            # (otherwise idle) gpsimd queue, w2 behind the constants on the
            # scalar queue — the first L1 unit needs w1+x complete, so two
            # parallel transfer chains halve the time to first compute
            w1t = w1_pool.tile([128, KT1, HID], bf16, tag="w1", name="w1t")
            nc.gpsimd.dma_start(
                out=w1t, in_=w1_[t].rearrange("(k p) h -> p k h", p=128))
            xt = xt_pool.tile([128, KT1, NP], bf16, tag="xt", name="xt")
            nc.sync.dma_start(
                out=xt, in_=xd_[t].rearrange("(k p) n -> p k n", p=128))
            w2t = w2_pool.tile([128, KT2, DG], bf16, tag="w2", name="w2t")
            nc.scalar.dma_start(
                out=w2t, in_=w2_[t].rearrange("(k p) d -> p k d", p=128))
            return w1t, xt, w2t

        def emit_l1_unit(t, w1t, xt, m, h_sb):
            ct = CQ if t == "q" else CK
            nt = NQ if t == "q" else NP
            p1 = big.tile([128, NPB], f32, tag="big", name="p1")
            for k in range(KT1):
                for c0, cs in ct:
                    nc.tensor.matmul(
                        p1[:, c0:c0 + cs],
                        w1t[:, k, m * 128:(m + 1) * 128],
                        xt[:, k, c0:c0 + cs],
                        start=(k == 0), stop=(k == KT1 - 1),
                    )
            ht = h_pool.tile([128, NP], bf16, tag="h", name="ht")
            nc.scalar.activation(
                out=ht[:, :nt], in_=p1[:, :nt], func=AF.Tanh,
                bias=bpk_sb[:, B1OFF[t] + m:B1OFF[t] + m + 1], scale=1.0,
            )
            h_sb.append(ht)

        def emit_l2_unit(t, w2t, h_sb, m):
            ct = CQ if t == "q" else CK
            nt = NQ if t == "q" else NP
            dst = qT if t == "q" else kT
            p2 = big.tile([128, NPB], f32, tag="big", name="p2")
            for k in range(KT2):
                for c0, cs in ct:
                    nc.tensor.matmul(
                        p2[:, c0:c0 + cs],
                        w2t[:, k, m * 128:(m + 1) * 128],
                        h_sb[k][:, c0:c0 + cs],
                        start=(k == 0), stop=(k == KT2 - 1),
                    )
            nc.scalar.activation(
                out=dst[:, m, :nt], in_=p2[:, :nt], func=AF.Identity,
                bias=bpk_sb[:, B2OFF[t] + m:B2OFF[t] + m + 1], scale=1.0,
            )

        # window 1: k-L1 (scalar-bound on tanh; nothing to interleave)
        w1k, xk, w2k = emit_type_dma("k")
        h_k = []
        for m in range(MT1):
            emit_l1_unit("k", w1k, xk, m, h_k)
        # window 2: q-L1 with k-L2 interleaved (front-loaded so the h(k)
        # tiles are fully read before the h-pool rotation reuses them)
        w1q, xq, w2q = emit_type_dma("q")
        h_q = []
        plan2 = [("l2", 0), ("l1", 0), ("l1", 1), ("l2", 1), ("l1", 2),
                 ("l1", 3), ("l2", 2), ("l2", 3), ("l1", 4), ("l1", 5),
                 ("l1", 6), ("l1", 7)]
        for kind, m in plan2:
            if kind == "l1":
                emit_l1_unit("q", w1q, xq, m, h_q)
            else:
                emit_l2_unit("k", w2k, h_k, m)
        # window 3: v-L1 with q-L2 interleaved
        w1v, xv, w2v = emit_type_dma("v")
        h_v = []
        for kind, m in plan2:
            if kind == "l1":
                emit_l1_unit("v", w1v, xv, m, h_v)
            else:
                emit_l2_unit("q", w2q, h_q, m)


        ysc = [ysc_pool.tile([128, NP], bf16, tag="ysc", name=f"ysc{i}")
               for i in range(HEADS_G)]

        def emit_y2_group(hd, c0, cs):
            y2c = small.tile([128, 512], f32, tag="small")
            for kt in range(NTOK):
                vt = v_sb[kt // 4][
                    :, (kt % 4) * DG + hd * 128:
                    (kt % 4) * DG + (hd + 1) * 128]
                nc.tensor.matmul(
                    y2c[:, :cs], vt, pts[(hd, kt)][:, c0:c0 + cs],
                    start=(kt == 0), stop=(kt == NTOK - 1),
                )
            nc.vector.tensor_tensor(
                ysc[hd][:, c0:c0 + cs], y2c[:, :cs], rb[hd][:, c0:c0 + cs],
                ALU.mult)

        # ---------------- phase 2: v-L2 + S-units of heads 0..2 ----------
        # The v-L2 matmul stream (9 token tiles x (8 accum + 1 bias) = 81
        # matmuls) is interleaved 3-per-S-unit so the tensor queue always
        # has exp-independent work between the exp-gated S^T tiles.
        pv_cur = [None]

        def emit_v_mm(idx):
            tt, j = idx // (KT2 + 1), idx % (KT2 + 1)
            if j == 0:
                pv_cur[0] = small.tile([128, 512], f32, tag="small",
                                       name="pv")
            if j < KT2:
                nc.tensor.matmul(
                    pv_cur[0][:, :],
                    h_v[j][:, tt * 128:(tt + 1) * 128],
                    w2v[:, j, :],
                    start=(j == 0), stop=False,
                )
            else:
                nc.tensor.matmul(
                    pv_cur[0][:, :], e0_sb[:, :], bv2_sb[:, :],
                    start=False, stop=True,
                )
                nc.vector.tensor_copy(
                    out=v_sb[tt // 4][:, (tt % 4) * DG:(tt % 4 + 1) * DG],
                    in_=pv_cur[0][:, :],
                )

        s_units = [(hd, kt) for hd in range(HEADS_G - 1)
                   for kt in range(NTOK)]
        nvm = NTOK * (KT2 + 1)
        vi = 0
        pend_aux = []
        for i, (hd, kt) in enumerate(s_units):
            emit_s_unit(hd, kt)
            if kt == NTOK - 1:
                pend_aux.append((hd, i))
            if pend_aux and i >= pend_aux[0][1] + 2:
                emit_aux(pend_aux.pop(0)[0])
            vt_end = min(nvm, (nvm * (i + 1) + len(s_units) - 1)
                         // len(s_units))
            while vi < vt_end:
                emit_v_mm(vi)
                vi += 1
        while vi < nvm:
            emit_v_mm(vi)
            vi += 1

        # ---------------- phase 3: S(h3) + y2 groups + projection --------
        dma_eng = [nc.sync, nc.gpsimd]

        def emit_proj(od, c0, cs):
            pp = small.tile([128, 512], f32, tag="small", name="pp")
            for hd in range(HEADS_G):
                nc.tensor.matmul(
                    pp[:, :cs],
                    wp_sb[:, hd, od * 128:(od + 1) * 128],
                    ysc[hd][:, c0:c0 + cs],
                    start=(hd == 0), stop=(hd == HEADS_G - 1),
                )
            ot = out_pool.tile([128, 512], bf16, tag="out", name="ot")
            if od == 0:
                nc.scalar.activation(
                    out=ot[:, :cs], in_=pp[:, :cs], func=AF.Copy, scale=1.0)
            else:
                nc.vector.tensor_copy(out=ot[:, :cs], in_=pp[:, :cs])
            dma_eng[od].dma_start(
                out=outT[od * 128:(od + 1) * 128, c0:c0 + cs],
                in_=ot[:, :cs],
            )

        # y2-group order: the narrow tail chunk first per head, so a full
        # 512-wide group lands right before aux(h3) and covers the wait
        # for head 3's denominator running-sum to finish
        cq_ord = ([CQ[-1]] + list(CQ[:-1])) if len(CQ) > 1 else list(CQ)
        groups = [(hd, c0, cs) for hd in range(HEADS_G - 1)
                  for c0, cs in cq_ord]
        h3 = HEADS_G - 1
        emit_s_unit(h3, 0)
        gi = 0
        for kt in range(1, NTOK):
            if gi < len(groups):
                emit_y2_group(*groups[gi])
                gi += 1
            emit_s_unit(h3, kt)
            if kt == 3 and pend_aux:
                emit_aux(pend_aux.pop(0)[0])
        while gi < len(groups):
            emit_y2_group(*groups[gi])
            gi += 1
        # h3 denominators, then y2(h3) column groups with the projection
        # matmuls (and their output DMAs) interleaved right behind them
        emit_aux(h3)
        emit_y2_group(h3, *CQ[0])
        for ci in range(1, len(CQ)):
            emit_y2_group(h3, *CQ[ci])
            emit_proj(0, *CQ[ci - 1])
            emit_proj(1, *CQ[ci - 1])
        emit_proj(0, *CQ[-1])
        emit_proj(1, *CQ[-1])

    nc.compile()
    return nc


def _perm_np(mask_b):
    """Valid-first stable permutation and valid count for one batch."""
    maskf = mask_b.astype(np.float32)
    perm = np.argsort(1.0 - maskf, kind="stable")
    nv = int(maskf.sum())
    return perm, nv


def _pad_tokens(x, NP):
    """x: (N, F) -> (NP, F) zero-padded/truncated token dim."""
    out = np.zeros((NP, x.shape[1]), np.float32)
    n = min(NP, x.shape[0])
    out[:n] = x[:n]
    return out


def _prep_core_inputs(inputs, b, g, NP):
    import ml_dtypes

    f32 = np.float32
    bf = ml_dtypes.bfloat16
    sl = slice(g * DG, (g + 1) * DG)
    scale = float(Dh) ** -0.5
    perm, nv = _perm_np(inputs["mask"][b, :, 0])
    km = np.full(NP, NEG, f32)
    km[:nv] = 0.0
    e0 = np.zeros((128, 128), f32)
    e0[0, :] = 1.0
    eyeC = np.ones((128, 128), f32) - np.eye(128, dtype=f32)
    bv2r = np.zeros((128, DG), f32)
    bv2r[0] = inputs["bv2"][sl].astype(f32)
    # bias pack: [b1v | b1k | b1q | b2q | b2k]  (cols 0:8, 8:16, 16:24,
    # 24:28, 28:32); b1 columns are the per-m-tile partition biases.
    bpk = np.zeros((128, 32), f32)
    bpk[:, 0:8] = inputs["bv1"].astype(f32).reshape(HID // 128, 128).T
    bpk[:, 8:16] = inputs["bk1"].astype(f32).reshape(HID // 128, 128).T
    bpk[:, 16:24] = inputs["bq1"].astype(f32).reshape(HID // 128, 128).T
    bpk[:, 24:28] = (inputs["bq2"][sl].astype(f32) * scale).reshape(
        DG // 128, 128).T
    bpk[:, 28:32] = inputs["bk2"][sl].astype(f32).reshape(DG // 128, 128).T

    def ptok(x):   # permute tokens valid-first, pad to NP
        return _pad_tokens(x[perm].astype(f32), NP)

    return {
        "xqT": np.ascontiguousarray(ptok(inputs["query"][b]).T).astype(bf),
        "xkT": np.ascontiguousarray(ptok(inputs["key"][b]).T).astype(bf),
        "xvT": np.ascontiguousarray(ptok(inputs["value"][b]).T).astype(bf),
        "wq1": np.ascontiguousarray(inputs["Wq1"].astype(bf)),
        "wk1": np.ascontiguousarray(inputs["Wk1"].astype(bf)),
        "wv1": np.ascontiguousarray(inputs["Wv1"].astype(bf)),
        "wq2": np.ascontiguousarray(
            (inputs["Wq2"][:, sl].astype(f32) * scale).astype(bf)),
        "wk2": np.ascontiguousarray(inputs["Wk2"][:, sl].astype(bf)),
        "wv2": np.ascontiguousarray(inputs["Wv2"][:, sl].astype(bf)),
        "bpk": bpk,
        "bv2row": bv2r.astype(bf),
        "e0d": e0.astype(bf),
        "onesd": np.ones((128, 128), bf),
        "eyeCd": eyeC.astype(bf),
        "kmd": np.ascontiguousarray(km.reshape(NP // 128, 128).T),
        "wpb": np.ascontiguousarray(inputs["Wp"][sl, :].astype(bf)),
    }


def kernel(**inputs):
    import sys
    if "/opt/trn_rl_repo" not in sys.path:
        sys.path.insert(0, "/opt/trn_rl_repo")
    from concourse.bass_utils import run_bass_kernel_spmd

    inputs = {k: np.asarray(v) for k, v in inputs.items()}

    nv_max = int(inputs["mask"][:, :, 0].sum(axis=1).max())
    NP = ((nv_max + 127) // 128) * 128
    NQ = ((nv_max + 63) // 64) * 64   # query width: valid queries only

    if _CACHE.get("NP") != NP or _CACHE.get("NQ") != NQ:
        _CACHE["nc"] = _build_nc(NP, NQ)
        _CACHE["NP"] = NP
        _CACHE["NQ"] = NQ
    nc = _CACHE["nc"]

    in_maps = [
        _prep_core_inputs(inputs, c // HG, c % HG, NP) for c in range(NCORES)
    ]

    res = run_bass_kernel_spmd(nc, in_maps, core_ids=list(range(NCORES)))
    results = res.results

    bp = inputs["bp"].astype(np.float32)
    out = np.empty((B, N, OUT_DIM), np.float32)
    for b in range(B):
        acc = results[b * HG]["outT"].astype(np.float32)
        for g in range(1, HG):
            acc = acc + results[b * HG + g]["outT"].astype(np.float32)
        perm, nv = _perm_np(inputs["mask"][b, :, 0])
        out[b] = bp[None, :]
        out[b, perm[:nv]] = acc.T[:nv] + bp[None, :]
    return out


# revision 50
# speedup vs baseline: 1.0126x; 1.0126x over previous
"""Bass/Trainium2 kernel for nn_Attention (B=4, N=2048, IN=256, HID=1024,
D=1024, OUT=256, H=8 heads), SPMD over 8 NeuronCores.

Sharding: core c handles batch b = c//2 and head-group g = c%2 (4 heads,
512 of the 1024 inner features).  Layer-1 of each QKV MLP is recomputed on
both cores of a batch (cheap); the output projection is computed per
head-group and the two partial products are summed on the host (plus bias).

Mask compaction: ~half the tokens are masked out (key mask) and masked
queries only output the bias row.  The host applies ONE permutation
(valid tokens first) to q, k and v inputs, so the kernel runs on
NP = ceil(max_valid/128)*128 tokens instead of N=2048.  Padded key rows
get an additive -30000 before exp (as the per-partition Exp bias).

All matmuls run in bf16.  The query axis is additionally trimmed to
NQ = ceil(max_valid/64)*64 columns (padded queries are discarded on the
host, so nothing reads them).  DMA triggers cost ~0.6us each on their
issuing queue, so inputs are fetched as ONE merged tile per tensor, the
big w1/x tiles stream on the sync queue in first-use order (k's are
split in half so compute starts earlier), w2/constants ride the scalar
queue, and tiny bias tables are packed into one [128,32] tile.  A short
garbage-data matmul warmup ramps the PE p-state (full clock needs ~3us
of continuous execution, and any tensor-queue gap drops it back for
~3us) while the first input tiles stream in; the whole schedule is built
to keep the tensor queue gap-free.

Schedule (the Exp stream on the Scalar engine is the attention limiter,
so three heads' score/exp work runs inside the v-L2 window where Scalar
is otherwise idle):
  1. k-L1; then q-L1 with k-L2 interleaved; then v-L1 with q-L2
     interleaved (keeps tanh/Identity off the critical path)
  2. the 81 v-L2 matmuls interleaved 3-per-S-unit with the 27 S-units of
     heads 0..2
     (S-unit kt: S^T tile [128,NQ] = kT_kt.T @ qT via chunk matmuls;
      Exp with key-mask partition bias -> pt bf16; diagonal zeroed on
      GPSIMD (pt *= 1-I); denominator running-sum on DVE; per head,
      deferred 2 S-units past its last exp: per-chunk all-ones
      stationary matmul -> broadcast sums -> DVE reciprocal_approx_fast
      -> rb[hd])
  3. S-units of head 3 (front-loaded by one slot) interleaved with the
     y2-groups of heads 0..2
     (y2-group (hd,c): 9 accumulating AV matmuls into a 1-bank PSUM
      chunk, then ysc[hd][:,c] = y2c * rb[hd][:,c] on DVE), then
     y2-groups of head 3 with the projection matmuls and bf16 output
     copies/DMAs interleaved right behind them.

PSUM: "big" pool 2 x 3 banks (L1/L2 accumulators and S^T tiles), "small"
pool 2 x 1 bank (warmup, v-L2 tiles, denominator chunks, y2 chunks,
projection).
"""

import numpy as np

B, N, IN_DIM, HID, D, OUT_DIM, H = 4, 2048, 256, 1024, 1024, 256, 8
NCORES = 8
HG = 2                 # head groups (cores per batch)
DG = D // HG           # 512 features per group
HEADS_G = H // HG      # 4 heads per core
Dh = D // H            # 128
NEG = -30000.0         # additive mask value (exp underflows to 0)

_CACHE = {}


def _chunks(total, size):
    out = []
    o = 0
    while o < total:
        s = min(size, total - o)
        out.append((o, s))
        o += s
    return out


def _build_nc(NP, NQ):
    import concourse.mybir as mybir
    import concourse.tile as tile
    from concourse import bacc
    from contextlib import ExitStack

    dt = mybir.dt
    f32 = dt.float32
    bf16 = dt.bfloat16
    AF = mybir.ActivationFunctionType
    ALU = mybir.AluOpType

    # Keep all used activation funcs (Tanh, Exp) in ONE table set so the
    # table-load pass never thrashes.
    if not getattr(bacc, "_act_tables_patched", False):
        from concourse import hw_specs as _hw
        _orig_get = _hw.get_activation_tables

        def _patched(arch):
            tables = dict(_orig_get(arch))
            AFT = mybir.ActivationFunctionType
            keep = {"exp_and_others", "natural_log_exp_and_others"}
            for name in tables:
                if name in keep:
                    continue
                fns = tables[name]
                if AFT.Exp in fns or AFT.Ln in fns:
                    tables[name] = set()
            return tables

        _patched.__wrapped__ = _orig_get
        bacc.get_activation_tables = _patched
        bacc._act_tables_patched = True

    nc = bacc.Bacc("TRN2", target_bir_lowering=False, debug=False)

    # ---- DRAM I/O ----
    xd_ = {}
    w1_ = {}
    w2_ = {}
    for t in ("k", "q", "v"):
        xd_[t] = nc.dram_tensor(f"x{t}T", [IN_DIM, NP], bf16,
                                kind="ExternalInput")
        w1_[t] = nc.dram_tensor(f"w{t}1", [IN_DIM, HID], bf16,
                                kind="ExternalInput")
        w2_[t] = nc.dram_tensor(f"w{t}2", [HID, DG], bf16,
                                kind="ExternalInput")
    bpk = nc.dram_tensor("bpk", [128, 32], f32, kind="ExternalInput")
    bv2row = nc.dram_tensor("bv2row", [128, DG], bf16, kind="ExternalInput")
    e0d = nc.dram_tensor("e0d", [128, 128], bf16, kind="ExternalInput")
    onesd = nc.dram_tensor("onesd", [128, 128], bf16, kind="ExternalInput")
    eyeCd = nc.dram_tensor("eyeCd", [128, 128], bf16, kind="ExternalInput")
    kmd = nc.dram_tensor("kmd", [128, NP // 128], f32, kind="ExternalInput")
    wpb = nc.dram_tensor("wpb", [DG, OUT_DIM], bf16, kind="ExternalInput")
    outT = nc.dram_tensor("outT", [OUT_DIM, NP], bf16, kind="ExternalOutput")

    KT1 = IN_DIM // 128          # 2  k-tiles in layer 1
    KT2 = HID // 128             # 8  k-tiles in layer 2
    MT1 = HID // 128             # 8  m-tiles in layer 1
    NTOK = NP // 128             # key-token tiles
    CK = _chunks(NP, 512)        # key/value token chunks (bank-aligned)
    CQ = _chunks(NQ, 512)        # query token chunks (trimmed to valid)
    NPB = ((NP + 511) // 512) * 512   # psum cols rounded to full banks
    # bias-pack column offsets: b1 per type (8 each), then b2q, b2k (4 each)
    B1OFF = {"v": 0, "k": 8, "q": 16}
    B2OFF = {"q": 24, "k": 28}

    with tile.TileContext(nc) as tc, ExitStack() as ctx:
        # PSUM: big = 2 x 3 banks, small = 2 x 1 bank  (8 banks total)
        big = ctx.enter_context(tc.tile_pool(name="big", bufs=2,
                                             space="PSUM"))
        small = ctx.enter_context(tc.tile_pool(name="small", bufs=2,
                                               space="PSUM"))
        singles = ctx.enter_context(tc.tile_pool(name="singles", bufs=1))
        xt_pool = ctx.enter_context(tc.tile_pool(name="xt", bufs=2))
        w1_pool = ctx.enter_context(tc.tile_pool(name="w1", bufs=2))
        w2_pool = ctx.enter_context(tc.tile_pool(name="w2", bufs=2))
        h_pool = ctx.enter_context(tc.tile_pool(name="h", bufs=12))
        qk_pool = ctx.enter_context(tc.tile_pool(name="qk", bufs=2))
        v_pool = ctx.enter_context(
            tc.tile_pool(name="v", bufs=(NTOK + 3) // 4))
        pt_pool = ctx.enter_context(tc.tile_pool(name="pt", bufs=28))
        sacc_pool = ctx.enter_context(tc.tile_pool(name="sacc", bufs=2))
        rb_pool = ctx.enter_context(tc.tile_pool(name="rb", bufs=3))
        ysc_pool = ctx.enter_context(tc.tile_pool(name="ysc", bufs=4))
        out_pool = ctx.enter_context(tc.tile_pool(name="out", bufs=4))

        # ---- warmup: ramp the PE p-state on zeroed garbage data (the PE
        # needs ~3us of continuous execution to reach full clock; any idle
        # gap drops it back for the next ~3us, so the schedule below is
        # built to keep the tensor queue gap-free) ----
        wu = singles.tile([128, 512], bf16, tag="wu")
        nc.gpsimd.memset(wu[:, :], 0)
        wups = small.tile([128, 512], f32, tag="small")
        for _ in range(12):
            nc.tensor.matmul(wups[:, :], wu[:, :128], wu[:, :],
                             start=True, stop=True)
        nc.vector.tensor_copy(out=wu[:, 0:1], in_=wups[:, 0:1])

        # ---- small constants on the scalar queue (idle at start); w2
        # weights also go there per-type so the sync queue streams only
        # the critical w1/x tiles in first-use order ----
        bpk_sb = singles.tile([128, 32], f32, tag="bpk")
        nc.scalar.dma_start(out=bpk_sb, in_=bpk[:, :])
        ones_sb = singles.tile([128, 128], bf16, tag="ones")
        nc.scalar.dma_start(out=ones_sb, in_=onesd[:, :])
        eyeC_sb = singles.tile([128, 128], bf16, tag="eyeC")
        nc.scalar.dma_start(out=eyeC_sb, in_=eyeCd[:, :])
        km_sb = singles.tile([128, NP // 128], f32, tag="km")
        nc.scalar.dma_start(out=km_sb, in_=kmd[:, :])
        wp_sb = singles.tile([128, HEADS_G, OUT_DIM], bf16, tag="wp")
        nc.scalar.dma_start(
            out=wp_sb, in_=wpb.rearrange("(h p) o -> p h o", p=128))
        bv2_sb = singles.tile([128, DG], bf16, tag="bv2")
        nc.gpsimd.dma_start(out=bv2_sb, in_=bv2row[:, :])
        e0_sb = singles.tile([128, 128], bf16, tag="e0")
        nc.gpsimd.dma_start(out=e0_sb, in_=e0d[:, :])

        # persistent activations
        qT = qk_pool.tile([128, HEADS_G, NP], bf16, tag="qk", name="qT")
        kT = qk_pool.tile([128, HEADS_G, NP], bf16, tag="qk", name="kT")
        v_sb = [v_pool.tile([128, 4 * DG], bf16, tag="v", name=f"v{i}")
                for i in range((NTOK + 3) // 4)]

        # ---- S-unit / denominator helpers (used from phase 1 onward) ----
        pts = {}
        rb = {}
        saccs = {}
        sacc_cur = [None]

        def emit_s_unit(hd, kt):
            st = big.tile([128, NPB], f32, tag="big", name="st")
            for c0, cs in CQ:
                nc.tensor.matmul(
                    st[:, c0:c0 + cs],
                    kT[:, hd, kt * 128:(kt + 1) * 128],
                    qT[:, hd, c0:c0 + cs],
                    start=True, stop=True,
                )
            pt = pt_pool.tile([128, NP], bf16, tag="pt", name="pt")
            nc.scalar.activation(
                out=pt[:, :NQ], in_=st[:, :NQ], func=AF.Exp,
                bias=km_sb[:, kt:kt + 1], scale=1.0,
            )
            # no self-attention: zero the diagonal block on GPSIMD
            db = kt * 128
            dw = min(128, NQ - db)
            if dw > 0:
                nc.gpsimd.tensor_tensor(
                    pt[:, db:db + dw], pt[:, db:db + dw], eyeC_sb[:, :dw],
                    ALU.mult)
            if kt == 0:
                sacc_cur[0] = sacc_pool.tile([128, NP], bf16, tag="sacc",
                                             name="sacc")
                nc.vector.tensor_copy(out=sacc_cur[0][:, :NQ],
                                      in_=pt[:, :NQ])
            else:
                nc.vector.tensor_tensor(sacc_cur[0][:, :NQ],
                                        sacc_cur[0][:, :NQ], pt[:, :NQ],
                                        ALU.add)
            pts[(hd, kt)] = pt
            if kt == NTOK - 1:
                saccs[hd] = sacc_cur[0]

        def emit_aux(hd):
            # denominators -> broadcast sums -> 1/s.  Deferred a couple of
            # S-units past the head's last exp so the tensor queue never
            # stalls waiting for the DVE running-sum chain to finish.
            rbt = rb_pool.tile([128, NP], f32, tag="rb", name="rbt")
            for c0, cs in CQ:
                aux = small.tile([128, 512], f32, tag="small", name="aux")
                nc.tensor.matmul(
                    aux[:, :cs], ones_sb[:, :], saccs[hd][:, c0:c0 + cs],
                    start=True, stop=True,
                )
                nc.vector.reciprocal_approx_fast(
                    out=rbt[:, c0:c0 + cs], in_=aux[:, :cs])
            rb[hd] = rbt

        # ---------------- phase 1: k-MLP, q-MLP, v-L1 --------------------
        # Six S-units of head 0 are interleaved into q-L2 (the Scalar
        # engine has tanh-free slack there), shrinking phase 2's exp floor.
        def emit_type_dma(t):
            w1t = w1_pool.tile([128, KT1, HID], bf16, tag="w1", name="w1t")
            w1ap = w1_[t].rearrange("(k p) h -> p k h", p=128)
            xt = xt_pool.tile([128, KT1, NP], bf16, tag="xt", name="xt")
            xap = xd_[t].rearrange("(k p) n -> p k n", p=128)
            if t == "k":
                # split the first tiles so compute can start after ~half
                # the bytes have landed (first-use order on the sync queue)
                nc.sync.dma_start(out=w1t[:, :, :HID // 2],
                                  in_=w1ap[:, :, :HID // 2])
                nc.sync.dma_start(out=xt[:, 0, :], in_=xap[:, 0, :])
                nc.sync.dma_start(out=xt[:, 1, :], in_=xap[:, 1, :])
                nc.sync.dma_start(out=w1t[:, :, HID // 2:],
                                  in_=w1ap[:, :, HID // 2:])
            else:
                nc.sync.dma_start(out=w1t, in_=w1ap)
                nc.sync.dma_start(out=xt, in_=xap)
            w2t = w2_pool.tile([128, KT2, DG], bf16, tag="w2", name="w2t")
            nc.scalar.dma_start(
                out=w2t, in_=w2_[t].rearrange("(k p) d -> p k d", p=128))
            return w1t, xt, w2t

        def emit_l1_unit(t, w1t, xt, m, h_sb):
            ct = CQ if t == "q" else CK
            nt = NQ if t == "q" else NP
            p1 = big.tile([128, NPB], f32, tag="big", name="p1")
            for k in range(KT1):
                for c0, cs in ct:
                    nc.tensor.matmul(
                        p1[:, c0:c0 + cs],
                        w1t[:, k, m * 128:(m + 1) * 128],
                        xt[:, k, c0:c0 + cs],
                        start=(k == 0), stop=(k == KT1 - 1),
                    )
            ht = h_pool.tile([128, NP], bf16, tag="h", name="ht")
            nc.scalar.activation(
                out=ht[:, :nt], in_=p1[:, :nt], func=AF.Tanh,
                bias=bpk_sb[:, B1OFF[t] + m:B1OFF[t] + m + 1], scale=1.0,
            )
            h_sb.append(ht)

        def emit_l2_unit(t, w2t, h_sb, m):
            ct = CQ if t == "q" else CK
            nt = NQ if t == "q" else NP
            dst = qT if t == "q" else kT
            p2 = big.tile([128, NPB], f32, tag="big", name="p2")
            for k in range(KT2):
                for c0, cs in ct:
                    nc.tensor.matmul(
                        p2[:, c0:c0 + cs],
                        w2t[:, k, m * 128:(m + 1) * 128],
                        h_sb[k][:, c0:c0 + cs],
                        start=(k == 0), stop=(k == KT2 - 1),
                    )
            nc.scalar.activation(
                out=dst[:, m, :nt], in_=p2[:, :nt], func=AF.Identity,
                bias=bpk_sb[:, B2OFF[t] + m:B2OFF[t] + m + 1], scale=1.0,
            )

        # window 1: k-L1 (scalar-bound on tanh; nothing to interleave)
        w1k, xk, w2k = emit_type_dma("k")
        h_k = []
        for m in range(MT1):
            emit_l1_unit("k", w1k, xk, m, h_k)
        # window 2: q-L1 with k-L2 interleaved (front-loaded so the h(k)
        # tiles are fully read before the h-pool rotation reuses them)
        w1q, xq, w2q = emit_type_dma("q")
        h_q = []
        plan2 = [("l2", 0), ("l1", 0), ("l1", 1), ("l2", 1), ("l1", 2),
                 ("l1", 3), ("l2", 2), ("l2", 3), ("l1", 4), ("l1", 5),
                 ("l1", 6), ("l1", 7)]
        for kind, m in plan2:
            if kind == "l1":
                emit_l1_unit("q", w1q, xq, m, h_q)
            else:
                emit_l2_unit("k", w2k, h_k, m)
        # window 3: v-L1 with q-L2 interleaved
        w1v, xv, w2v = emit_type_dma("v")
        h_v = []
        for kind, m in plan2:
            if kind == "l1":
                emit_l1_unit("v", w1v, xv, m, h_v)
            else:
                emit_l2_unit("q", w2q, h_q, m)


        ysc = [ysc_pool.tile([128, NP], bf16, tag="ysc", name=f"ysc{i}")
               for i in range(HEADS_G)]

        def emit_y2_group(hd, c0, cs):
            y2c = small.tile([128, 512], f32, tag="small")
            for kt in range(NTOK):
                vt = v_sb[kt // 4][
                    :, (kt % 4) * DG + hd * 128:
                    (kt % 4) * DG + (hd + 1) * 128]
                nc.tensor.matmul(
                    y2c[:, :cs], vt, pts[(hd, kt)][:, c0:c0 + cs],
                    start=(kt == 0), stop=(kt == NTOK - 1),
                )
            nc.vector.tensor_tensor(
                ysc[hd][:, c0:c0 + cs], y2c[:, :cs], rb[hd][:, c0:c0 + cs],
                ALU.mult)

        # ---------------- phase 2: v-L2 + S-units of heads 0..2 ----------
        # The v-L2 matmul stream (9 token tiles x (8 accum + 1 bias) = 81
        # matmuls) is interleaved 3-per-S-unit so the tensor queue always
        # has exp-independent work between the exp-gated S^T tiles.
        pv_cur = [None]

        def emit_v_mm(idx):
            tt, j = idx // (KT2 + 1), idx % (KT2 + 1)
            if j == 0:
                pv_cur[0] = small.tile([128, 512], f32, tag="small",
                                       name="pv")
            if j < KT2:
                nc.tensor.matmul(
                    pv_cur[0][:, :],
                    h_v[j][:, tt * 128:(tt + 1) * 128],
                    w2v[:, j, :],
                    start=(j == 0), stop=False,
                )
            else:
                nc.tensor.matmul(
                    pv_cur[0][:, :], e0_sb[:, :], bv2_sb[:, :],
                    start=False, stop=True,
                )
                nc.vector.tensor_copy(
                    out=v_sb[tt // 4][:, (tt % 4) * DG:(tt % 4 + 1) * DG],
                    in_=pv_cur[0][:, :],
                )

        s_units = [(hd, kt) for hd in range(HEADS_G - 1)
                   for kt in range(NTOK)]
        nvm = NTOK * (KT2 + 1)
        vi = 0
        pend_aux = []
        for i, (hd, kt) in enumerate(s_units):
            emit_s_unit(hd, kt)
            if kt == NTOK - 1:
                pend_aux.append((hd, i))
            if pend_aux and i >= pend_aux[0][1] + 2:
                emit_aux(pend_aux.pop(0)[0])
            vt_end = min(nvm, (nvm * (i + 1) + len(s_units) - 1)
                         // len(s_units))
            while vi < vt_end:
                emit_v_mm(vi)
                vi += 1
        while vi < nvm:
            emit_v_mm(vi)
            vi += 1

        # ---------------- phase 3: S(h3) + y2 groups + projection --------
        dma_eng = [nc.sync, nc.gpsimd]

        def emit_proj(od, c0, cs):
            pp = small.tile([128, 512], f32, tag="small", name="pp")
            for hd in range(HEADS_G):
                nc.tensor.matmul(
                    pp[:, :cs],
                    wp_sb[:, hd, od * 128:(od + 1) * 128],
                    ysc[hd][:, c0:c0 + cs],
                    start=(hd == 0), stop=(hd == HEADS_G - 1),
                )
            ot = out_pool.tile([128, 512], bf16, tag="out", name="ot")
            if od == 0:
                nc.scalar.activation(
                    out=ot[:, :cs], in_=pp[:, :cs], func=AF.Copy, scale=1.0)
            else:
                nc.vector.tensor_copy(out=ot[:, :cs], in_=pp[:, :cs])
            dma_eng[od].dma_start(
                out=outT[od * 128:(od + 1) * 128, c0:c0 + cs],
                in_=ot[:, :cs],
            )

        # y2-group order: the narrow tail chunk first per head, so a full
        # 512-wide group lands right before aux(h3) and covers the wait
        # for head 3's denominator running-sum to finish
        cq_ord = ([CQ[-1]] + list(CQ[:-1])) if len(CQ) > 1 else list(CQ)
        groups = [(hd, c0, cs) for hd in range(HEADS_G - 1)
                  for c0, cs in cq_ord]
        h3 = HEADS_G - 1
        emit_s_unit(h3, 0)
        gi = 0
        for kt in range(1, NTOK):
            if gi < len(groups):
                emit_y2_group(*groups[gi])
                gi += 1
            emit_s_unit(h3, kt)
            if kt == 3 and pend_aux:
                emit_aux(pend_aux.pop(0)[0])
        while gi < len(groups):
            emit_y2_group(*groups[gi])
            gi += 1
        # h3 denominators, then y2(h3) column groups with the projection
        # matmuls (and their output DMAs) interleaved right behind them
        emit_aux(h3)
        emit_y2_group(h3, *CQ[0])
        for ci in range(1, len(CQ)):
            emit_y2_group(h3, *CQ[ci])
            emit_proj(0, *CQ[ci - 1])
            emit_proj(1, *CQ[ci - 1])
        emit_proj(0, *CQ[-1])
        emit_proj(1, *CQ[-1])

    nc.compile()
    return nc


def _perm_np(mask_b):
    """Valid-first stable permutation and valid count for one batch."""
    maskf = mask_b.astype(np.float32)
    perm = np.argsort(1.0 - maskf, kind="stable")
    nv = int(maskf.sum())
    return perm, nv


def _pad_tokens(x, NP):
    """x: (N, F) -> (NP, F) zero-padded/truncated token dim."""
    out = np.zeros((NP, x.shape[1]), np.float32)
    n = min(NP, x.shape[0])
    out[:n] = x[:n]
    return out


def _prep_core_inputs(inputs, b, g, NP):
    import ml_dtypes

    f32 = np.float32
    bf = ml_dtypes.bfloat16
    sl = slice(g * DG, (g + 1) * DG)
    scale = float(Dh) ** -0.5
    perm, nv = _perm_np(inputs["mask"][b, :, 0])
    km = np.full(NP, NEG, f32)
    km[:nv] = 0.0
    e0 = np.zeros((128, 128), f32)
    e0[0, :] = 1.0
    eyeC = np.ones((128, 128), f32) - np.eye(128, dtype=f32)
    bv2r = np.zeros((128, DG), f32)
    bv2r[0] = inputs["bv2"][sl].astype(f32)
    # bias pack: [b1v | b1k | b1q | b2q | b2k]  (cols 0:8, 8:16, 16:24,
    # 24:28, 28:32); b1 columns are the per-m-tile partition biases.
    bpk = np.zeros((128, 32), f32)
    bpk[:, 0:8] = inputs["bv1"].astype(f32).reshape(HID // 128, 128).T
    bpk[:, 8:16] = inputs["bk1"].astype(f32).reshape(HID // 128, 128).T
    bpk[:, 16:24] = inputs["bq1"].astype(f32).reshape(HID // 128, 128).T
    bpk[:, 24:28] = (inputs["bq2"][sl].astype(f32) * scale).reshape(
        DG // 128, 128).T
    bpk[:, 28:32] = inputs["bk2"][sl].astype(f32).reshape(DG // 128, 128).T

    def ptok(x):   # permute tokens valid-first, pad to NP
        return _pad_tokens(x[perm].astype(f32), NP)

    return {
        "xqT": np.ascontiguousarray(ptok(inputs["query"][b]).T).astype(bf),
        "xkT": np.ascontiguousarray(ptok(inputs["key"][b]).T).astype(bf),
        "xvT": np.ascontiguousarray(ptok(inputs["value"][b]).T).astype(bf),
        "wq1": np.ascontiguousarray(inputs["Wq1"].astype(bf)),
        "wk1": np.ascontiguousarray(inputs["Wk1"].astype(bf)),
        "wv1": np.ascontiguousarray(inputs["Wv1"].astype(bf)),
        "wq2": np.ascontiguousarray(
            (inputs["Wq2"][:, sl].astype(f32) * scale).astype(bf)),
        "wk2": np.ascontiguousarray(inputs["Wk2"][:, sl].astype(bf)),
        "wv2": np.ascontiguousarray(inputs["Wv2"][:, sl].astype(bf)),
        "bpk": bpk,
        "bv2row": bv2r.astype(bf),
        "e0d": e0.astype(bf),
        "onesd": np.ones((128, 128), bf),
        "eyeCd": eyeC.astype(bf),
        "kmd": np.ascontiguousarray(km.reshape(NP // 128, 128).T),
        "wpb": np.ascontiguousarray(inputs["Wp"][sl, :].astype(bf)),
    }


def kernel(**inputs):
    import sys
    if "/opt/trn_rl_repo" not in sys.path:
        sys.path.insert(0, "/opt/trn_rl_repo")
    from concourse.bass_utils import run_bass_kernel_spmd

    inputs = {k: np.asarray(v) for k, v in inputs.items()}

    nv_max = int(inputs["mask"][:, :, 0].sum(axis=1).max())
    NP = ((nv_max + 127) // 128) * 128
    NQ = ((nv_max + 63) // 64) * 64   # query width: valid queries only

    if _CACHE.get("NP") != NP or _CACHE.get("NQ") != NQ:
        _CACHE["nc"] = _build_nc(NP, NQ)
        _CACHE["NP"] = NP
        _CACHE["NQ"] = NQ
    nc = _CACHE["nc"]

    in_maps = [
        _prep_core_inputs(inputs, c // HG, c % HG, NP) for c in range(NCORES)
    ]

    res = run_bass_kernel_spmd(nc, in_maps, core_ids=list(range(NCORES)))
    results = res.results

    bp = inputs["bp"].astype(np.float32)
    out = np.empty((B, N, OUT_DIM), np.float32)
    for b in range(B):
        acc = results[b * HG]["outT"].astype(np.float32)
        for g in range(1, HG):
            acc = acc + results[b * HG + g]["outT"].astype(np.float32)
        perm, nv = _perm_np(inputs["mask"][b, :, 0])
        out[b] = bp[None, :]
        out[b, perm[:nv]] = acc.T[:nv] + bp[None, :]
    return out


# revision 53
# speedup vs baseline: 1.0128x; 1.0002x over previous
"""Bass/Trainium2 kernel for nn_Attention (B=4, N=2048, IN=256, HID=1024,
D=1024, OUT=256, H=8 heads), SPMD over 8 NeuronCores.

Sharding: core c handles batch b = c//2 and head-group g = c%2 (4 heads,
512 of the 1024 inner features).  Layer-1 of each QKV MLP is recomputed on
both cores of a batch (cheap); the output projection is computed per
head-group and the two partial products are summed on the host (plus bias).

Mask compaction: ~half the tokens are masked out (key mask) and masked
queries only output the bias row.  The host applies ONE permutation
(valid tokens first) to q, k and v inputs, so the kernel runs on
NP = ceil(max_valid/128)*128 tokens instead of N=2048.  Padded key rows
get an additive -30000 before exp (as the per-partition Exp bias).

All matmuls run in bf16.  The query axis is additionally trimmed to
NQ = ceil(max_valid/64)*64 columns (padded queries are discarded on the
host, so nothing reads them).  DMA triggers cost ~0.6us each on their
issuing queue, so inputs are fetched as ONE merged tile per tensor, the
big w1/x tiles stream on the sync queue in first-use order (k's are
split in half so compute starts earlier), w2/constants ride the scalar
queue, and tiny bias tables are packed into one [128,32] tile.  A short
garbage-data matmul warmup ramps the PE p-state (full clock needs ~3us
of continuous execution, and any tensor-queue gap drops it back for
~3us) while the first input tiles stream in; the whole schedule is built
to keep the tensor queue gap-free.

Schedule (the Exp stream on the Scalar engine is the attention limiter,
so three heads' score/exp work runs inside the v-L2 window where Scalar
is otherwise idle):
  1. k-L1; then q-L1 with k-L2 interleaved; then v-L1 with q-L2
     interleaved (keeps tanh/Identity off the critical path)
  2. the 81 v-L2 matmuls interleaved 3-per-S-unit with the 27 S-units of
     heads 0..2
     (S-unit kt: S^T tile [128,NQ] = kT_kt.T @ qT via chunk matmuls;
      Exp with key-mask partition bias -> pt bf16; diagonal zeroed on
      GPSIMD (pt *= 1-I); denominator running-sum on DVE; per head,
      deferred 2 S-units past its last exp: per-chunk all-ones
      stationary matmul -> broadcast sums -> DVE reciprocal_approx_fast
      -> rb[hd])
  3. S-units of head 3 (front-loaded by one slot) interleaved with the
     y2-groups of heads 0..2
     (y2-group (hd,c): 9 accumulating AV matmuls into a 1-bank PSUM
      chunk, then ysc[hd][:,c] = y2c * rb[hd][:,c] on DVE), then
     y2-groups of head 3 with the projection matmuls and bf16 output
     copies/DMAs interleaved right behind them.

PSUM: "big" pool 2 x 3 banks (L1/L2 accumulators and S^T tiles), "small"
pool 2 x 1 bank (warmup, v-L2 tiles, denominator chunks, y2 chunks,
projection).
"""

import numpy as np

B, N, IN_DIM, HID, D, OUT_DIM, H = 4, 2048, 256, 1024, 1024, 256, 8
NCORES = 8
HG = 2                 # head groups (cores per batch)
DG = D // HG           # 512 features per group
HEADS_G = H // HG      # 4 heads per core
Dh = D // H            # 128
NEG = -30000.0         # additive mask value (exp underflows to 0)

_CACHE = {}


def _chunks(total, size):
    out = []
    o = 0
    while o < total:
        s = min(size, total - o)
        out.append((o, s))
        o += s
    return out


def _build_nc(NP, NQ):
    import concourse.mybir as mybir
    import concourse.tile as tile
    from concourse import bacc
    from contextlib import ExitStack

    dt = mybir.dt
    f32 = dt.float32
    bf16 = dt.bfloat16
    AF = mybir.ActivationFunctionType
    ALU = mybir.AluOpType

    # Keep all used activation funcs (Tanh, Exp) in ONE table set so the
    # table-load pass never thrashes.
    if not getattr(bacc, "_act_tables_patched", False):
        from concourse import hw_specs as _hw
        _orig_get = _hw.get_activation_tables

        def _patched(arch):
            tables = dict(_orig_get(arch))
            AFT = mybir.ActivationFunctionType
            keep = {"exp_and_others", "natural_log_exp_and_others"}
            for name in tables:
                if name in keep:
                    continue
                fns = tables[name]
                if AFT.Exp in fns or AFT.Ln in fns:
                    tables[name] = set()
            return tables

        _patched.__wrapped__ = _orig_get
        bacc.get_activation_tables = _patched
        bacc._act_tables_patched = True

    nc = bacc.Bacc("TRN2", target_bir_lowering=False, debug=False)

    # ---- DRAM I/O ----
    xd_ = {}
    w1_ = {}
    w2_ = {}
    for t in ("k", "q", "v"):
        xd_[t] = nc.dram_tensor(f"x{t}T", [IN_DIM, NP], bf16,
                                kind="ExternalInput")
        w1_[t] = nc.dram_tensor(f"w{t}1", [IN_DIM, HID], bf16,
                                kind="ExternalInput")
        w2_[t] = nc.dram_tensor(f"w{t}2", [HID, DG], bf16,
                                kind="ExternalInput")
    bpk = nc.dram_tensor("bpk", [128, 32], f32, kind="ExternalInput")
    bv2row = nc.dram_tensor("bv2row", [128, DG], bf16, kind="ExternalInput")
    e0d = nc.dram_tensor("e0d", [128, 128], bf16, kind="ExternalInput")
    onesd = nc.dram_tensor("onesd", [128, 128], bf16, kind="ExternalInput")
    eyeCd = nc.dram_tensor("eyeCd", [128, 128], bf16, kind="ExternalInput")
    kmd = nc.dram_tensor("kmd", [128, NP // 128], f32, kind="ExternalInput")
    wpb = nc.dram_tensor("wpb", [DG, OUT_DIM], bf16, kind="ExternalInput")
    outT = nc.dram_tensor("outT", [OUT_DIM, NP], bf16, kind="ExternalOutput")

    KT1 = IN_DIM // 128          # 2  k-tiles in layer 1
    KT2 = HID // 128             # 8  k-tiles in layer 2
    MT1 = HID // 128             # 8  m-tiles in layer 1
    NTOK = NP // 128             # key-token tiles
    CK = _chunks(NP, 512)        # key/value token chunks (bank-aligned)
    CQ = _chunks(NQ, 512)        # query token chunks (trimmed to valid)
    NPB = ((NP + 511) // 512) * 512   # psum cols rounded to full banks
    # bias-pack column offsets: b1 per type (8 each), then b2q, b2k (4 each)
    B1OFF = {"v": 0, "k": 8, "q": 16}
    B2OFF = {"q": 24, "k": 28}

    with tile.TileContext(nc) as tc, ExitStack() as ctx:
        # PSUM: big = 2 x 3 banks, small = 2 x 1 bank  (8 banks total)
        big = ctx.enter_context(tc.tile_pool(name="big", bufs=2,
                                             space="PSUM"))
        small = ctx.enter_context(tc.tile_pool(name="small", bufs=2,
                                               space="PSUM"))
        singles = ctx.enter_context(tc.tile_pool(name="singles", bufs=1))
        xt_pool = ctx.enter_context(tc.tile_pool(name="xt", bufs=2))
        w1_pool = ctx.enter_context(tc.tile_pool(name="w1", bufs=2))
        w2_pool = ctx.enter_context(tc.tile_pool(name="w2", bufs=2))
        h_pool = ctx.enter_context(tc.tile_pool(name="h", bufs=12))
        qk_pool = ctx.enter_context(tc.tile_pool(name="qk", bufs=2))
        v_pool = ctx.enter_context(
            tc.tile_pool(name="v", bufs=(NTOK + 3) // 4))
        pt_pool = ctx.enter_context(tc.tile_pool(name="pt", bufs=28))
        sacc_pool = ctx.enter_context(tc.tile_pool(name="sacc", bufs=2))
        rb_pool = ctx.enter_context(tc.tile_pool(name="rb", bufs=3))
        ysc_pool = ctx.enter_context(tc.tile_pool(name="ysc", bufs=4))
        out_pool = ctx.enter_context(tc.tile_pool(name="out", bufs=4))

        # ---- warmup: ramp the PE p-state on zeroed garbage data (the PE
        # needs ~3us of continuous execution to reach full clock; any idle
        # gap drops it back for the next ~3us, so the schedule below is
        # built to keep the tensor queue gap-free) ----
        wu = singles.tile([128, 512], bf16, tag="wu")
        nc.gpsimd.memset(wu[:, :], 0)
        wups = small.tile([128, 512], f32, tag="small")
        for _ in range(12):
            nc.tensor.matmul(wups[:, :], wu[:, :128], wu[:, :],
                             start=True, stop=True)
        nc.vector.tensor_copy(out=wu[:, 0:1], in_=wups[:, 0:1])

        # ---- small constants on the scalar queue (idle at start); w2
        # weights also go there per-type so the sync queue streams only
        # the critical w1/x tiles in first-use order ----
        bpk_sb = singles.tile([128, 32], f32, tag="bpk")
        nc.scalar.dma_start(out=bpk_sb, in_=bpk[:, :])
        ones_sb = singles.tile([128, 128], bf16, tag="ones")
        nc.scalar.dma_start(out=ones_sb, in_=onesd[:, :])
        eyeC_sb = singles.tile([128, 128], bf16, tag="eyeC")
        nc.scalar.dma_start(out=eyeC_sb, in_=eyeCd[:, :])
        km_sb = singles.tile([128, NP // 128], f32, tag="km")
        nc.scalar.dma_start(out=km_sb, in_=kmd[:, :])
        wp_sb = singles.tile([128, HEADS_G, OUT_DIM], bf16, tag="wp")
        nc.scalar.dma_start(
            out=wp_sb, in_=wpb.rearrange("(h p) o -> p h o", p=128))
        bv2_sb = singles.tile([128, DG], bf16, tag="bv2")
        nc.gpsimd.dma_start(out=bv2_sb, in_=bv2row[:, :])
        e0_sb = singles.tile([128, 128], bf16, tag="e0")
        nc.gpsimd.dma_start(out=e0_sb, in_=e0d[:, :])

        # persistent activations
        qT = qk_pool.tile([128, HEADS_G, NP], bf16, tag="qk", name="qT")
        kT = qk_pool.tile([128, HEADS_G, NP], bf16, tag="qk", name="kT")
        v_sb = [v_pool.tile([128, 4 * DG], bf16, tag="v", name=f"v{i}")
                for i in range((NTOK + 3) // 4)]

        # ---- S-unit / denominator helpers (used from phase 1 onward) ----
        pts = {}
        rb = {}
        saccs = {}
        sacc_cur = [None]

        def emit_s_unit(hd, kt):
            st = big.tile([128, NPB], f32, tag="big", name="st")
            for c0, cs in CQ:
                nc.tensor.matmul(
                    st[:, c0:c0 + cs],
                    kT[:, hd, kt * 128:(kt + 1) * 128],
                    qT[:, hd, c0:c0 + cs],
                    start=True, stop=True,
                )
            pt = pt_pool.tile([128, NP], bf16, tag="pt", name="pt")
            nc.scalar.activation(
                out=pt[:, :NQ], in_=st[:, :NQ], func=AF.Exp,
                bias=km_sb[:, kt:kt + 1], scale=1.0,
            )
            # no self-attention: zero the diagonal block on GPSIMD
            db = kt * 128
            dw = min(128, NQ - db)
            if dw > 0:
                nc.gpsimd.tensor_tensor(
                    pt[:, db:db + dw], pt[:, db:db + dw], eyeC_sb[:, :dw],
                    ALU.mult)
            if kt == 0:
                sacc_cur[0] = sacc_pool.tile([128, NP], bf16, tag="sacc",
                                             name="sacc")
                nc.vector.tensor_copy(out=sacc_cur[0][:, :NQ],
                                      in_=pt[:, :NQ])
            else:
                nc.vector.tensor_tensor(sacc_cur[0][:, :NQ],
                                        sacc_cur[0][:, :NQ], pt[:, :NQ],
                                        ALU.add)
            pts[(hd, kt)] = pt
            if kt == NTOK - 1:
                saccs[hd] = sacc_cur[0]

        def emit_aux(hd):
            # denominators -> broadcast sums -> 1/s.  Deferred a couple of
            # S-units past the head's last exp so the tensor queue never
            # stalls waiting for the DVE running-sum chain to finish.
            rbt = rb_pool.tile([128, NP], f32, tag="rb", name="rbt")
            for c0, cs in CQ:
                aux = small.tile([128, 512], f32, tag="small", name="aux")
                nc.tensor.matmul(
                    aux[:, :cs], ones_sb[:, :], saccs[hd][:, c0:c0 + cs],
                    start=True, stop=True,
                )
                nc.vector.reciprocal_approx_fast(
                    out=rbt[:, c0:c0 + cs], in_=aux[:, :cs])
            rb[hd] = rbt

        # ---------------- phase 1: k-MLP, q-MLP, v-L1 --------------------
        # Six S-units of head 0 are interleaved into q-L2 (the Scalar
        # engine has tanh-free slack there), shrinking phase 2's exp floor.
        def emit_type_dma(t):
            w1t = w1_pool.tile([128, KT1, HID], bf16, tag="w1", name="w1t")
            w1ap = w1_[t].rearrange("(k p) h -> p k h", p=128)
            xt = xt_pool.tile([128, KT1, NP], bf16, tag="xt", name="xt")
            xap = xd_[t].rearrange("(k p) n -> p k n", p=128)
            if t == "k":
                # split the first tiles so compute can start after ~half
                # the bytes have landed (first-use order on the sync queue)
                nc.sync.dma_start(out=w1t[:, :, :HID // 2],
                                  in_=w1ap[:, :, :HID // 2])
                nc.sync.dma_start(out=xt[:, 0, :], in_=xap[:, 0, :])
                nc.sync.dma_start(out=xt[:, 1, :], in_=xap[:, 1, :])
                nc.sync.dma_start(out=w1t[:, :, HID // 2:],
                                  in_=w1ap[:, :, HID // 2:])
            else:
                nc.sync.dma_start(out=w1t, in_=w1ap)
                nc.sync.dma_start(out=xt, in_=xap)
            w2t = w2_pool.tile([128, KT2, DG], bf16, tag="w2", name="w2t")
            nc.scalar.dma_start(
                out=w2t, in_=w2_[t].rearrange("(k p) d -> p k d", p=128))
            return w1t, xt, w2t

        def emit_l1_unit(t, w1t, xt, m, h_sb):
            ct = CQ if t == "q" else CK
            nt = NQ if t == "q" else NP
            p1 = big.tile([128, NPB], f32, tag="big", name="p1")
            for k in range(KT1):
                for c0, cs in ct:
                    nc.tensor.matmul(
                        p1[:, c0:c0 + cs],
                        w1t[:, k, m * 128:(m + 1) * 128],
                        xt[:, k, c0:c0 + cs],
                        start=(k == 0), stop=(k == KT1 - 1),
                    )
            ht = h_pool.tile([128, NP], bf16, tag="h", name="ht")
            nc.scalar.activation(
                out=ht[:, :nt], in_=p1[:, :nt], func=AF.Tanh,
                bias=bpk_sb[:, B1OFF[t] + m:B1OFF[t] + m + 1], scale=1.0,
            )
            h_sb.append(ht)

        def emit_l2_unit(t, w2t, h_sb, m):
            ct = CQ if t == "q" else CK
            nt = NQ if t == "q" else NP
            dst = qT if t == "q" else kT
            p2 = big.tile([128, NPB], f32, tag="big", name="p2")
            for k in range(KT2):
                for c0, cs in ct:
                    nc.tensor.matmul(
                        p2[:, c0:c0 + cs],
                        w2t[:, k, m * 128:(m + 1) * 128],
                        h_sb[k][:, c0:c0 + cs],
                        start=(k == 0), stop=(k == KT2 - 1),
                    )
            nc.scalar.activation(
                out=dst[:, m, :nt], in_=p2[:, :nt], func=AF.Identity,
                bias=bpk_sb[:, B2OFF[t] + m:B2OFF[t] + m + 1], scale=1.0,
            )

        # window 1: k-L1 (scalar-bound on tanh; nothing to interleave)
        w1k, xk, w2k = emit_type_dma("k")
        h_k = []
        for m in range(MT1):
            emit_l1_unit("k", w1k, xk, m, h_k)
        # window 2: q-L1 with k-L2 interleaved (front-loaded so the h(k)
        # tiles are fully read before the h-pool rotation reuses them)
        w1q, xq, w2q = emit_type_dma("q")
        h_q = []
        plan2 = [("l2", 0), ("l1", 0), ("l1", 1), ("l2", 1), ("l1", 2),
                 ("l1", 3), ("l2", 2), ("l2", 3), ("l1", 4), ("l1", 5),
                 ("l1", 6), ("l1", 7)]
        for kind, m in plan2:
            if kind == "l1":
                emit_l1_unit("q", w1q, xq, m, h_q)
            else:
                emit_l2_unit("k", w2k, h_k, m)
        # window 3: v-L1 with q-L2 interleaved
        w1v, xv, w2v = emit_type_dma("v")
        h_v = []
        for kind, m in plan2:
            if kind == "l1":
                emit_l1_unit("v", w1v, xv, m, h_v)
            else:
                emit_l2_unit("q", w2q, h_q, m)


        ysc = [ysc_pool.tile([128, NP], bf16, tag="ysc", name=f"ysc{i}")
               for i in range(HEADS_G)]

        def emit_y2_group(hd, c0, cs):
            y2c = small.tile([128, 512], f32, tag="small")
            for kt in range(NTOK):
                vt = v_sb[kt // 4][
                    :, (kt % 4) * DG + hd * 128:
                    (kt % 4) * DG + (hd + 1) * 128]
                nc.tensor.matmul(
                    y2c[:, :cs], vt, pts[(hd, kt)][:, c0:c0 + cs],
                    start=(kt == 0), stop=(kt == NTOK - 1),
                )
            nc.vector.tensor_tensor(
                ysc[hd][:, c0:c0 + cs], y2c[:, :cs], rb[hd][:, c0:c0 + cs],
                ALU.mult)

        # ---------------- phase 2: v-L2 + S-units of heads 0..2 ----------
        # The v-L2 matmul stream (9 token tiles x (8 accum + 1 bias) = 81
        # matmuls) is interleaved 3-per-S-unit so the tensor queue always
        # has exp-independent work between the exp-gated S^T tiles.
        pv_cur = [None]

        def emit_v_mm(idx):
            tt, j = idx // (KT2 + 1), idx % (KT2 + 1)
            if j == 0:
                pv_cur[0] = small.tile([128, 512], f32, tag="small",
                                       name="pv")
            if j < KT2:
                nc.tensor.matmul(
                    pv_cur[0][:, :],
                    h_v[j][:, tt * 128:(tt + 1) * 128],
                    w2v[:, j, :],
                    start=(j == 0), stop=False,
                )
            else:
                nc.tensor.matmul(
                    pv_cur[0][:, :], e0_sb[:, :], bv2_sb[:, :],
                    start=False, stop=True,
                )
                nc.vector.tensor_copy(
                    out=v_sb[tt // 4][:, (tt % 4) * DG:(tt % 4 + 1) * DG],
                    in_=pv_cur[0][:, :],
                )

        s_units = [(hd, kt) for hd in range(HEADS_G - 1)
                   for kt in range(NTOK)]
        nvm = NTOK * (KT2 + 1)
        vi = 0
        pend_aux = []
        for i, (hd, kt) in enumerate(s_units):
            emit_s_unit(hd, kt)
            if kt == NTOK - 1:
                pend_aux.append((hd, i))
            if pend_aux and i >= pend_aux[0][1] + 2:
                emit_aux(pend_aux.pop(0)[0])
            vt_end = min(nvm, (nvm * (i + 1) + len(s_units) - 1)
                         // len(s_units))
            while vi < vt_end:
                emit_v_mm(vi)
                vi += 1
        while vi < nvm:
            emit_v_mm(vi)
            vi += 1

        # ---------------- phase 3: S(h3) + y2 groups + projection --------
        dma_eng = [nc.sync, nc.gpsimd]

        def emit_proj(od, c0, cs):
            pp = small.tile([128, 512], f32, tag="small", name="pp")
            for hd in range(HEADS_G):
                nc.tensor.matmul(
                    pp[:, :cs],
                    wp_sb[:, hd, od * 128:(od + 1) * 128],
                    ysc[hd][:, c0:c0 + cs],
                    start=(hd == 0), stop=(hd == HEADS_G - 1),
                )
            ot = out_pool.tile([128, 512], bf16, tag="out", name="ot")
            if od == 0:
                nc.scalar.activation(
                    out=ot[:, :cs], in_=pp[:, :cs], func=AF.Copy, scale=1.0)
            else:
                nc.vector.tensor_copy(out=ot[:, :cs], in_=pp[:, :cs])
            dma_eng[od].dma_start(
                out=outT[od * 128:(od + 1) * 128, c0:c0 + cs],
                in_=ot[:, :cs],
            )

        # y2-group order: the narrow tail chunk first per head, so a full
        # 512-wide group lands right before aux(h3) and covers the wait
        # for head 3's denominator running-sum to finish
        cq_ord = ([CQ[-1]] + list(CQ[:-1])) if len(CQ) > 1 else list(CQ)
        groups = [(hd, c0, cs) for hd in range(HEADS_G - 1)
                  for c0, cs in cq_ord]
        h3 = HEADS_G - 1
        emit_s_unit(h3, 0)
        gi = 0
        for kt in range(1, NTOK):
            if gi < len(groups):
                emit_y2_group(*groups[gi])
                gi += 1
            emit_s_unit(h3, kt)
            if kt == 3 and pend_aux:
                emit_aux(pend_aux.pop(0)[0])
        while gi < len(groups):
            emit_y2_group(*groups[gi])
            gi += 1
        # h3 denominators, then y2(h3) column groups with the projection
        # matmuls (and their output DMAs) interleaved right behind them
        emit_aux(h3)
        emit_y2_group(h3, *CQ[0])
        for ci in range(1, len(CQ)):
            emit_y2_group(h3, *CQ[ci])
            emit_proj(0, *CQ[ci - 1])
            emit_proj(1, *CQ[ci - 1])
        emit_proj(0, *CQ[-1])
        emit_proj(1, *CQ[-1])

    nc.compile()
    return nc


def _perm_np(mask_b):
    """Valid-first stable permutation and valid count for one batch."""
    maskf = mask_b.astype(np.float32)
    perm = np.argsort(1.0 - maskf, kind="stable")
    nv = int(maskf.sum())
    return perm, nv


def _pad_tokens(x, NP):
    """x: (N, F) -> (NP, F) zero-padded/truncated token dim."""
    out = np.zeros((NP, x.shape[1]), np.float32)
    n = min(NP, x.shape[0])
    out[:n] = x[:n]
    return out


def _prep_core_inputs(inputs, b, g, NP):
    import ml_dtypes

    f32 = np.float32
    bf = ml_dtypes.bfloat16
    sl = slice(g * DG, (g + 1) * DG)
    scale = float(Dh) ** -0.5
    perm, nv = _perm_np(inputs["mask"][b, :, 0])
    km = np.full(NP, NEG, f32)
    km[:nv] = 0.0
    e0 = np.zeros((128, 128), f32)
    e0[0, :] = 1.0
    eyeC = np.ones((128, 128), f32) - np.eye(128, dtype=f32)
    bv2r = np.zeros((128, DG), f32)
    bv2r[0] = inputs["bv2"][sl].astype(f32)
    # bias pack: [b1v | b1k | b1q | b2q | b2k]  (cols 0:8, 8:16, 16:24,
    # 24:28, 28:32); b1 columns are the per-m-tile partition biases.
    bpk = np.zeros((128, 32), f32)
    bpk[:, 0:8] = inputs["bv1"].astype(f32).reshape(HID // 128, 128).T
    bpk[:, 8:16] = inputs["bk1"].astype(f32).reshape(HID // 128, 128).T
    bpk[:, 16:24] = inputs["bq1"].astype(f32).reshape(HID // 128, 128).T
    bpk[:, 24:28] = (inputs["bq2"][sl].astype(f32) * scale).reshape(
        DG // 128, 128).T
    bpk[:, 28:32] = inputs["bk2"][sl].astype(f32).reshape(DG // 128, 128).T

    def ptok(x):   # permute tokens valid-first, pad to NP
        return _pad_tokens(x[perm].astype(f32), NP)

    return {
        "xqT": np.ascontiguousarray(ptok(inputs["query"][b]).T).astype(bf),
        "xkT": np.ascontiguousarray(ptok(inputs["key"][b]).T).astype(bf),
        "xvT": np.ascontiguousarray(ptok(inputs["value"][b]).T).astype(bf),
        "wq1": np.ascontiguousarray(inputs["Wq1"].astype(bf)),
        "wk1": np.ascontiguousarray(inputs["Wk1"].astype(bf)),
        "wv1": np.ascontiguousarray(inputs["Wv1"].astype(bf)),
        "wq2": np.ascontiguousarray(
            (inputs["Wq2"][:, sl].astype(f32) * scale).astype(bf)),
        "wk2": np.ascontiguousarray(inputs["Wk2"][:, sl].astype(bf)),
        "wv2": np.ascontiguousarray(inputs["Wv2"][:, sl].astype(bf)),
        "bpk": bpk,
        "bv2row": bv2r.astype(bf),
        "e0d": e0.astype(bf),
        "onesd": np.ones((128, 128), bf),
        "eyeCd": eyeC.astype(bf),
        "kmd": np.ascontiguousarray(km.reshape(NP // 128, 128).T),
        "wpb": np.ascontiguousarray(inputs["Wp"][sl, :].astype(bf)),
    }


def kernel(**inputs):
    import sys
    if "/opt/trn_rl_repo" not in sys.path:
        sys.path.insert(0, "/opt/trn_rl_repo")
    from concourse.bass_utils import run_bass_kernel_spmd

    inputs = {k: np.asarray(v) for k, v in inputs.items()}

    nv_max = int(inputs["mask"][:, :, 0].sum(axis=1).max())
    NP = ((nv_max + 127) // 128) * 128
    NQ = ((nv_max + 63) // 64) * 64   # query width: valid queries only

    if _CACHE.get("NP") != NP or _CACHE.get("NQ") != NQ:
        _CACHE["nc"] = _build_nc(NP, NQ)
        _CACHE["NP"] = NP
        _CACHE["NQ"] = NQ
    nc = _CACHE["nc"]

    in_maps = [
        _prep_core_inputs(inputs, c // HG, c % HG, NP) for c in range(NCORES)
    ]

    res = run_bass_kernel_spmd(nc, in_maps, core_ids=list(range(NCORES)))
    results = res.results

    bp = inputs["bp"].astype(np.float32)
    out = np.empty((B, N, OUT_DIM), np.float32)
    for b in range(B):
        acc = results[b * HG]["outT"].astype(np.float32)
        for g in range(1, HG):
            acc = acc + results[b * HG + g]["outT"].astype(np.float32)
        perm, nv = _perm_np(inputs["mask"][b, :, 0])
        out[b] = bp[None, :]
        out[b, perm[:nv]] = acc.T[:nv] + bp[None, :]
    return out


# revision 54
# speedup vs baseline: 1.0168x; 1.0039x over previous
"""Bass/Trainium2 kernel for nn_Attention (B=4, N=2048, IN=256, HID=1024,
D=1024, OUT=256, H=8 heads), SPMD over 8 NeuronCores.

Sharding: core c handles batch b = c//2 and head-group g = c%2 (4 heads,
512 of the 1024 inner features).  Layer-1 of each QKV MLP is recomputed on
both cores of a batch (cheap); the output projection is computed per
head-group and the two partial products are summed on the host (plus bias).

Mask compaction: ~half the tokens are masked out (key mask) and masked
queries only output the bias row.  The host applies ONE permutation
(valid tokens first) to q, k and v inputs, so the kernel runs on
NP = ceil(max_valid/128)*128 tokens instead of N=2048.  Padded key rows
get an additive -30000 before exp (as the per-partition Exp bias).

All matmuls run in bf16.  The query axis is additionally trimmed to
NQ = ceil(max_valid/64)*64 columns (padded queries are discarded on the
host, so nothing reads them).  DMA triggers cost ~0.6us each on their
issuing queue, so inputs are fetched as ONE merged tile per tensor, the
big w1/x tiles stream on the sync queue in first-use order (k's are
split in half so compute starts earlier), w2/constants ride the scalar
queue, and tiny bias tables are packed into one [128,32] tile.  A short
garbage-data matmul warmup ramps the PE p-state (full clock needs ~3us
of continuous execution, and any tensor-queue gap drops it back for
~3us) while the first input tiles stream in; the whole schedule is built
to keep the tensor queue gap-free.

Schedule (the Exp stream on the Scalar engine is the attention limiter,
so three heads' score/exp work runs inside the v-L2 window where Scalar
is otherwise idle):
  1. k-L1; then q-L1 with k-L2 interleaved; then v-L1 with q-L2
     interleaved (keeps tanh/Identity off the critical path)
  2. the 81 v-L2 matmuls interleaved 3-per-S-unit with the 27 S-units of
     heads 0..2
     (S-unit kt: S^T tile [128,NQ] = kT_kt.T @ qT via chunk matmuls;
      Exp with key-mask partition bias -> pt bf16; diagonal zeroed on
      GPSIMD (pt *= 1-I); denominator running-sum on DVE; per head,
      deferred 2 S-units past its last exp: per-chunk all-ones
      stationary matmul -> broadcast sums -> DVE reciprocal_approx_fast
      -> rb[hd])
  3. S-units of head 3 (front-loaded by one slot) interleaved with the
     y2-groups of heads 0..2
     (y2-group (hd,c): 9 accumulating AV matmuls into a 1-bank PSUM
      chunk, then ysc[hd][:,c] = y2c * rb[hd][:,c] on DVE), then
     y2-groups of head 3 with the projection matmuls and bf16 output
     copies/DMAs interleaved right behind them.

PSUM: "big" pool 2 x 3 banks (L1/L2 accumulators and S^T tiles), "small"
pool 2 x 1 bank (warmup, v-L2 tiles, denominator chunks, y2 chunks,
projection).
"""

import numpy as np

B, N, IN_DIM, HID, D, OUT_DIM, H = 4, 2048, 256, 1024, 1024, 256, 8
NCORES = 8
HG = 2                 # head groups (cores per batch)
DG = D // HG           # 512 features per group
HEADS_G = H // HG      # 4 heads per core
Dh = D // H            # 128
NEG = -30000.0         # additive mask value (exp underflows to 0)

_CACHE = {}


def _chunks(total, size):
    out = []
    o = 0
    while o < total:
        s = min(size, total - o)
        out.append((o, s))
        o += s
    return out


def _build_nc(NP, NQ):
    import concourse.mybir as mybir
    import concourse.tile as tile
    from concourse import bacc
    from contextlib import ExitStack

    dt = mybir.dt
    f32 = dt.float32
    bf16 = dt.bfloat16
    AF = mybir.ActivationFunctionType
    ALU = mybir.AluOpType

    # Keep all used activation funcs (Tanh, Exp) in ONE table set so the
    # table-load pass never thrashes.
    if not getattr(bacc, "_act_tables_patched", False):
        from concourse import hw_specs as _hw
        _orig_get = _hw.get_activation_tables

        def _patched(arch):
            tables = dict(_orig_get(arch))
            AFT = mybir.ActivationFunctionType
            keep = {"exp_and_others", "natural_log_exp_and_others"}
            for name in tables:
                if name in keep:
                    continue
                fns = tables[name]
                if AFT.Exp in fns or AFT.Ln in fns:
                    tables[name] = set()
            return tables

        _patched.__wrapped__ = _orig_get
        bacc.get_activation_tables = _patched
        bacc._act_tables_patched = True

    nc = bacc.Bacc("TRN2", target_bir_lowering=False, debug=False)

    # ---- DRAM I/O ----
    xd_ = {}
    w1_ = {}
    w2_ = {}
    for t in ("k", "q", "v"):
        xd_[t] = nc.dram_tensor(f"x{t}T", [IN_DIM, NP], bf16,
                                kind="ExternalInput")
        w1_[t] = nc.dram_tensor(f"w{t}1", [IN_DIM, HID], bf16,
                                kind="ExternalInput")
        w2_[t] = nc.dram_tensor(f"w{t}2", [HID, DG], bf16,
                                kind="ExternalInput")
    bpk = nc.dram_tensor("bpk", [128, 32], f32, kind="ExternalInput")
    bv2row = nc.dram_tensor("bv2row", [128, DG], bf16, kind="ExternalInput")
    e0d = nc.dram_tensor("e0d", [128, 128], bf16, kind="ExternalInput")
    onesd = nc.dram_tensor("onesd", [128, 128], bf16, kind="ExternalInput")
    eyeCd = nc.dram_tensor("eyeCd", [128, 128], bf16, kind="ExternalInput")
    kmd = nc.dram_tensor("kmd", [128, NP // 128], f32, kind="ExternalInput")
    wpb = nc.dram_tensor("wpb", [DG, OUT_DIM], bf16, kind="ExternalInput")
    outT = nc.dram_tensor("outT", [OUT_DIM, NP], bf16, kind="ExternalOutput")

    KT1 = IN_DIM // 128          # 2  k-tiles in layer 1
    KT2 = HID // 128             # 8  k-tiles in layer 2
    MT1 = HID // 128             # 8  m-tiles in layer 1
    NTOK = NP // 128             # key-token tiles
    CK = _chunks(NP, 512)        # key/value token chunks (bank-aligned)
    CQ = _chunks(NQ, 512)        # query token chunks (trimmed to valid)
    NPB = ((NP + 511) // 512) * 512   # psum cols rounded to full banks
    # bias-pack column offsets: b1 per type (8 each), then b2q, b2k (4 each)
    B1OFF = {"v": 0, "k": 8, "q": 16}
    B2OFF = {"q": 24, "k": 28}

    with tile.TileContext(nc) as tc, ExitStack() as ctx:
        # PSUM: big = 2 x 3 banks, small = 2 x 1 bank  (8 banks total)
        big = ctx.enter_context(tc.tile_pool(name="big", bufs=2,
                                             space="PSUM"))
        small = ctx.enter_context(tc.tile_pool(name="small", bufs=2,
                                               space="PSUM"))
        singles = ctx.enter_context(tc.tile_pool(name="singles", bufs=1))
        xt_pool = ctx.enter_context(tc.tile_pool(name="xt", bufs=2))
        w1_pool = ctx.enter_context(tc.tile_pool(name="w1", bufs=2))
        w2_pool = ctx.enter_context(tc.tile_pool(name="w2", bufs=2))
        h_pool = ctx.enter_context(tc.tile_pool(name="h", bufs=12))
        qk_pool = ctx.enter_context(tc.tile_pool(name="qk", bufs=2))
        v_pool = ctx.enter_context(
            tc.tile_pool(name="v", bufs=(NTOK + 3) // 4))
        pt_pool = ctx.enter_context(tc.tile_pool(name="pt", bufs=28))
        sacc_pool = ctx.enter_context(tc.tile_pool(name="sacc", bufs=2))
        rb_pool = ctx.enter_context(tc.tile_pool(name="rb", bufs=3))
        ysc_pool = ctx.enter_context(tc.tile_pool(name="ysc", bufs=4))
        out_pool = ctx.enter_context(tc.tile_pool(name="out", bufs=4))

        # ---- warmup: ramp the PE p-state on zeroed garbage data (the PE
        # needs ~3us of continuous execution to reach full clock; any idle
        # gap drops it back for the next ~3us, so the schedule below is
        # built to keep the tensor queue gap-free) ----
        wu = singles.tile([128, 512], bf16, tag="wu")
        nc.gpsimd.memset(wu[:, :], 0)
        wups = small.tile([128, 512], f32, tag="small")
        for _ in range(12):
            nc.tensor.matmul(wups[:, :], wu[:, :128], wu[:, :],
                             start=True, stop=True)
        nc.vector.tensor_copy(out=wu[:, 0:1], in_=wups[:, 0:1])

        # ---- small constants on the scalar queue (idle at start); w2
        # weights also go there per-type so the sync queue streams only
        # the critical w1/x tiles in first-use order ----
        bpk_sb = singles.tile([128, 32], f32, tag="bpk")
        nc.scalar.dma_start(out=bpk_sb, in_=bpk[:, :])
        ones_sb = singles.tile([128, 128], bf16, tag="ones")
        nc.scalar.dma_start(out=ones_sb, in_=onesd[:, :])
        eyeC_sb = singles.tile([128, 128], bf16, tag="eyeC")
        nc.scalar.dma_start(out=eyeC_sb, in_=eyeCd[:, :])
        km_sb = singles.tile([128, NP // 128], f32, tag="km")
        nc.scalar.dma_start(out=km_sb, in_=kmd[:, :])
        wp_sb = singles.tile([128, HEADS_G, OUT_DIM], bf16, tag="wp")
        nc.scalar.dma_start(
            out=wp_sb, in_=wpb.rearrange("(h p) o -> p h o", p=128))
        bv2_sb = singles.tile([128, DG], bf16, tag="bv2")
        nc.gpsimd.dma_start(out=bv2_sb, in_=bv2row[:, :])
        e0_sb = singles.tile([128, 128], bf16, tag="e0")
        nc.gpsimd.dma_start(out=e0_sb, in_=e0d[:, :])

        # persistent activations
        qT = qk_pool.tile([128, HEADS_G, NP], bf16, tag="qk", name="qT")
        kT = qk_pool.tile([128, HEADS_G, NP], bf16, tag="qk", name="kT")
        v_sb = [v_pool.tile([128, 4 * DG], bf16, tag="v", name=f"v{i}")
                for i in range((NTOK + 3) // 4)]

        # ---- S-unit / denominator helpers (used from phase 1 onward) ----
        pts = {}
        rb = {}
        saccs = {}
        sacc_cur = [None]

        def emit_s_unit(hd, kt):
            st = big.tile([128, NPB], f32, tag="big", name="st")
            for c0, cs in CQ:
                nc.tensor.matmul(
                    st[:, c0:c0 + cs],
                    kT[:, hd, kt * 128:(kt + 1) * 128],
                    qT[:, hd, c0:c0 + cs],
                    start=True, stop=True,
                )
            pt = pt_pool.tile([128, NP], bf16, tag="pt", name="pt")
            nc.scalar.activation(
                out=pt[:, :NQ], in_=st[:, :NQ], func=AF.Exp,
                bias=km_sb[:, kt:kt + 1], scale=1.0,
            )
            # no self-attention: zero the diagonal block on GPSIMD
            db = kt * 128
            dw = min(128, NQ - db)
            if dw > 0:
                nc.gpsimd.tensor_tensor(
                    pt[:, db:db + dw], pt[:, db:db + dw], eyeC_sb[:, :dw],
                    ALU.mult)
            if kt == 0:
                sacc_cur[0] = sacc_pool.tile([128, NP], bf16, tag="sacc",
                                             name="sacc")
                nc.vector.tensor_copy(out=sacc_cur[0][:, :NQ],
                                      in_=pt[:, :NQ])
            else:
                nc.vector.tensor_tensor(sacc_cur[0][:, :NQ],
                                        sacc_cur[0][:, :NQ], pt[:, :NQ],
                                        ALU.add)
            pts[(hd, kt)] = pt
            if kt == NTOK - 1:
                saccs[hd] = sacc_cur[0]

        def emit_aux(hd):
            # denominators -> broadcast sums -> 1/s.  Deferred a couple of
            # S-units past the head's last exp so the tensor queue never
            # stalls waiting for the DVE running-sum chain to finish.
            rbt = rb_pool.tile([128, NP], f32, tag="rb", name="rbt")
            for c0, cs in CQ:
                aux = small.tile([128, 512], f32, tag="small", name="aux")
                nc.tensor.matmul(
                    aux[:, :cs], ones_sb[:, :], saccs[hd][:, c0:c0 + cs],
                    start=True, stop=True,
                )
                nc.vector.reciprocal_approx_fast(
                    out=rbt[:, c0:c0 + cs], in_=aux[:, :cs])
            rb[hd] = rbt

        # ---------------- phase 1: k-MLP, q-MLP, v-L1 --------------------
        # Six S-units of head 0 are interleaved into q-L2 (the Scalar
        # engine has tanh-free slack there), shrinking phase 2's exp floor.
        def emit_type_dma(t):
            w1t = w1_pool.tile([128, KT1, HID], bf16, tag="w1", name="w1t")
            w1ap = w1_[t].rearrange("(k p) h -> p k h", p=128)
            xt = xt_pool.tile([128, KT1, NP], bf16, tag="xt", name="xt")
            xap = xd_[t].rearrange("(k p) n -> p k n", p=128)
            if t == "k":
                # split the first tiles so compute can start after ~half
                # the bytes have landed (first-use order on the sync queue)
                nc.sync.dma_start(out=w1t[:, :, :HID // 2],
                                  in_=w1ap[:, :, :HID // 2])
                nc.sync.dma_start(out=xt[:, 0, :], in_=xap[:, 0, :])
                nc.sync.dma_start(out=xt[:, 1, :], in_=xap[:, 1, :])
                nc.sync.dma_start(out=w1t[:, :, HID // 2:],
                                  in_=w1ap[:, :, HID // 2:])
            else:
                nc.sync.dma_start(out=w1t, in_=w1ap)
                nc.sync.dma_start(out=xt, in_=xap)
            w2t = w2_pool.tile([128, KT2, DG], bf16, tag="w2", name="w2t")
            nc.scalar.dma_start(
                out=w2t, in_=w2_[t].rearrange("(k p) d -> p k d", p=128))
            return w1t, xt, w2t

        def emit_l1_unit(t, w1t, xt, m, h_sb):
            ct = CQ if t == "q" else CK
            nt = NQ if t == "q" else NP
            p1 = big.tile([128, NPB], f32, tag="big", name="p1")
            for k in range(KT1):
                for c0, cs in ct:
                    nc.tensor.matmul(
                        p1[:, c0:c0 + cs],
                        w1t[:, k, m * 128:(m + 1) * 128],
                        xt[:, k, c0:c0 + cs],
                        start=(k == 0), stop=(k == KT1 - 1),
                    )
            ht = h_pool.tile([128, NP], bf16, tag="h", name="ht")
            nc.scalar.activation(
                out=ht[:, :nt], in_=p1[:, :nt], func=AF.Tanh,
                bias=bpk_sb[:, B1OFF[t] + m:B1OFF[t] + m + 1], scale=1.0,
            )
            h_sb.append(ht)

        def emit_l2_unit(t, w2t, h_sb, m):
            ct = CQ if t == "q" else CK
            nt = NQ if t == "q" else NP
            dst = qT if t == "q" else kT
            p2 = big.tile([128, NPB], f32, tag="big", name="p2")
            for k in range(KT2):
                for c0, cs in ct:
                    nc.tensor.matmul(
                        p2[:, c0:c0 + cs],
                        w2t[:, k, m * 128:(m + 1) * 128],
                        h_sb[k][:, c0:c0 + cs],
                        start=(k == 0), stop=(k == KT2 - 1),
                    )
            nc.scalar.activation(
                out=dst[:, m, :nt], in_=p2[:, :nt], func=AF.Identity,
                bias=bpk_sb[:, B2OFF[t] + m:B2OFF[t] + m + 1], scale=1.0,
            )

        # window 1: k-L1 (scalar-bound on tanh; nothing to interleave)
        w1k, xk, w2k = emit_type_dma("k")
        h_k = []
        for m in range(MT1):
            emit_l1_unit("k", w1k, xk, m, h_k)
        # window 2: q-L1 with k-L2 interleaved (front-loaded so the h(k)
        # tiles are fully read before the h-pool rotation reuses them)
        w1q, xq, w2q = emit_type_dma("q")
        h_q = []
        plan2 = [("l2", 0), ("l1", 0), ("l1", 1), ("l2", 1), ("l1", 2),
                 ("l1", 3), ("l2", 2), ("l2", 3), ("l1", 4), ("l1", 5),
                 ("l1", 6), ("l1", 7)]
        for kind, m in plan2:
            if kind == "l1":
                emit_l1_unit("q", w1q, xq, m, h_q)
            else:
                emit_l2_unit("k", w2k, h_k, m)
        # window 3: v-L1 with q-L2 interleaved
        w1v, xv, w2v = emit_type_dma("v")
        h_v = []
        for kind, m in plan2:
            if kind == "l1":
                emit_l1_unit("v", w1v, xv, m, h_v)
            else:
                emit_l2_unit("q", w2q, h_q, m)


        ysc = [ysc_pool.tile([128, NP], bf16, tag="ysc", name=f"ysc{i}")
               for i in range(HEADS_G)]

        def emit_y2_group(hd, c0, cs):
            y2c = small.tile([128, 512], f32, tag="small")
            for kt in range(NTOK):
                vt = v_sb[kt // 4][
                    :, (kt % 4) * DG + hd * 128:
                    (kt % 4) * DG + (hd + 1) * 128]
                nc.tensor.matmul(
                    y2c[:, :cs], vt, pts[(hd, kt)][:, c0:c0 + cs],
                    start=(kt == 0), stop=(kt == NTOK - 1),
                )
            nc.vector.tensor_tensor(
                ysc[hd][:, c0:c0 + cs], y2c[:, :cs], rb[hd][:, c0:c0 + cs],
                ALU.mult)

        # ---------------- phase 2: v-L2 + S-units of heads 0..2 ----------
        # The v-L2 matmul stream (9 token tiles x (8 accum + 1 bias) = 81
        # matmuls) is interleaved 3-per-S-unit so the tensor queue always
        # has exp-independent work between the exp-gated S^T tiles.
        pv_cur = [None]

        def emit_v_mm(idx):
            tt, j = idx // (KT2 + 1), idx % (KT2 + 1)
            if j == 0:
                pv_cur[0] = small.tile([128, 512], f32, tag="small",
                                       name="pv")
            if j < KT2:
                nc.tensor.matmul(
                    pv_cur[0][:, :],
                    h_v[j][:, tt * 128:(tt + 1) * 128],
                    w2v[:, j, :],
                    start=(j == 0), stop=False,
                )
            else:
                nc.tensor.matmul(
                    pv_cur[0][:, :], e0_sb[:, :], bv2_sb[:, :],
                    start=False, stop=True,
                )
                nc.vector.tensor_copy(
                    out=v_sb[tt // 4][:, (tt % 4) * DG:(tt % 4 + 1) * DG],
                    in_=pv_cur[0][:, :],
                )

        s_units = [(hd, kt) for hd in range(HEADS_G - 1)
                   for kt in range(NTOK)]
        nvm = NTOK * (KT2 + 1)
        vi = 0
        pend_aux = []
        for i, (hd, kt) in enumerate(s_units):
            if kt == 0 and i > 0:
                # pre-fill the head boundary: the first S^T tile of the new
                # head waits on the old head's trailing exps to free a PSUM
                # buf, so give the tensor queue v-L2 work to chew on first
                for _ in range(2):
                    if vi < nvm:
                        emit_v_mm(vi)
                        vi += 1
            emit_s_unit(hd, kt)
            if kt == NTOK - 1:
                pend_aux.append((hd, i))
            if pend_aux and i >= pend_aux[0][1] + 2:
                emit_aux(pend_aux.pop(0)[0])
            vt_end = min(nvm, (nvm * (i + 1) + len(s_units) - 1)
                         // len(s_units))
            while vi < vt_end:
                emit_v_mm(vi)
                vi += 1
        while vi < nvm:
            emit_v_mm(vi)
            vi += 1

        # ---------------- phase 3: S(h3) + y2 groups + projection --------
        dma_eng = [nc.sync, nc.gpsimd]

        def emit_proj(od, c0, cs):
            pp = small.tile([128, 512], f32, tag="small", name="pp")
            for hd in range(HEADS_G):
                nc.tensor.matmul(
                    pp[:, :cs],
                    wp_sb[:, hd, od * 128:(od + 1) * 128],
                    ysc[hd][:, c0:c0 + cs],
                    start=(hd == 0), stop=(hd == HEADS_G - 1),
                )
            ot = out_pool.tile([128, 512], bf16, tag="out", name="ot")
            if od == 0:
                nc.scalar.activation(
                    out=ot[:, :cs], in_=pp[:, :cs], func=AF.Copy, scale=1.0)
            else:
                nc.vector.tensor_copy(out=ot[:, :cs], in_=pp[:, :cs])
            dma_eng[od].dma_start(
                out=outT[od * 128:(od + 1) * 128, c0:c0 + cs],
                in_=ot[:, :cs],
            )

        # y2-group order: the narrow tail chunk first per head, so a full
        # 512-wide group lands right before aux(h3) and covers the wait
        # for head 3's denominator running-sum to finish
        cq_ord = ([CQ[-1]] + list(CQ[:-1])) if len(CQ) > 1 else list(CQ)
        groups = [(hd, c0, cs) for hd in range(HEADS_G - 1)
                  for c0, cs in cq_ord]
        h3 = HEADS_G - 1
        emit_s_unit(h3, 0)
        gi = 0
        for kt in range(1, NTOK):
            if gi < len(groups):
                emit_y2_group(*groups[gi])
                gi += 1
            emit_s_unit(h3, kt)
            if kt == 3 and pend_aux:
                emit_aux(pend_aux.pop(0)[0])
        while gi < len(groups):
            emit_y2_group(*groups[gi])
            gi += 1
        # h3 denominators, then y2(h3) column groups with the projection
        # matmuls (and their output DMAs) interleaved right behind them
        emit_aux(h3)
        emit_y2_group(h3, *CQ[0])
        for ci in range(1, len(CQ)):
            emit_y2_group(h3, *CQ[ci])
            emit_proj(0, *CQ[ci - 1])
            emit_proj(1, *CQ[ci - 1])
        emit_proj(0, *CQ[-1])
        emit_proj(1, *CQ[-1])

    nc.compile()
    return nc


def _perm_np(mask_b):
    """Valid-first stable permutation and valid count for one batch."""
    maskf = mask_b.astype(np.float32)
    perm = np.argsort(1.0 - maskf, kind="stable")
    nv = int(maskf.sum())
    return perm, nv


def _pad_tokens(x, NP):
    """x: (N, F) -> (NP, F) zero-padded/truncated token dim."""
    out = np.zeros((NP, x.shape[1]), np.float32)
    n = min(NP, x.shape[0])
    out[:n] = x[:n]
    return out


def _prep_core_inputs(inputs, b, g, NP):
    import ml_dtypes

    f32 = np.float32
    bf = ml_dtypes.bfloat16
    sl = slice(g * DG, (g + 1) * DG)
    scale = float(Dh) ** -0.5
    perm, nv = _perm_np(inputs["mask"][b, :, 0])
    km = np.full(NP, NEG, f32)
    km[:nv] = 0.0
    e0 = np.zeros((128, 128), f32)
    e0[0, :] = 1.0
    eyeC = np.ones((128, 128), f32) - np.eye(128, dtype=f32)
    bv2r = np.zeros((128, DG), f32)
    bv2r[0] = inputs["bv2"][sl].astype(f32)
    # bias pack: [b1v | b1k | b1q | b2q | b2k]  (cols 0:8, 8:16, 16:24,
    # 24:28, 28:32); b1 columns are the per-m-tile partition biases.
    bpk = np.zeros((128, 32), f32)
    bpk[:, 0:8] = inputs["bv1"].astype(f32).reshape(HID // 128, 128).T
    bpk[:, 8:16] = inputs["bk1"].astype(f32).reshape(HID // 128, 128).T
    bpk[:, 16:24] = inputs["bq1"].astype(f32).reshape(HID // 128, 128).T
    bpk[:, 24:28] = (inputs["bq2"][sl].astype(f32) * scale).reshape(
        DG // 128, 128).T
    bpk[:, 28:32] = inputs["bk2"][sl].astype(f32).reshape(DG // 128, 128).T

    def ptok(x):   # permute tokens valid-first, pad to NP
        return _pad_tokens(x[perm].astype(f32), NP)

    return {
        "xqT": np.ascontiguousarray(ptok(inputs["query"][b]).T).astype(bf),
        "xkT": np.ascontiguousarray(ptok(inputs["key"][b]).T).astype(bf),
        "xvT": np.ascontiguousarray(ptok(inputs["value"][b]).T).astype(bf),
        "wq1": np.ascontiguousarray(inputs["Wq1"].astype(bf)),
        "wk1": np.ascontiguousarray(inputs["Wk1"].astype(bf)),
        "wv1": np.ascontiguousarray(inputs["Wv1"].astype(bf)),
        "wq2": np.ascontiguousarray(
            (inputs["Wq2"][:, sl].astype(f32) * scale).astype(bf)),
        "wk2": np.ascontiguousarray(inputs["Wk2"][:, sl].astype(bf)),
        "wv2": np.ascontiguousarray(inputs["Wv2"][:, sl].astype(bf)),
        "bpk": bpk,
        "bv2row": bv2r.astype(bf),
        "e0d": e0.astype(bf),
        "onesd": np.ones((128, 128), bf),
        "eyeCd": eyeC.astype(bf),
        "kmd": np.ascontiguousarray(km.reshape(NP // 128, 128).T),
        "wpb": np.ascontiguousarray(inputs["Wp"][sl, :].astype(bf)),
    }


def kernel(**inputs):
    import sys
    if "/opt/trn_rl_repo" not in sys.path:
        sys.path.insert(0, "/opt/trn_rl_repo")
    from concourse.bass_utils import run_bass_kernel_spmd

    inputs = {k: np.asarray(v) for k, v in inputs.items()}

    nv_max = int(inputs["mask"][:, :, 0].sum(axis=1).max())
    NP = ((nv_max + 127) // 128) * 128
    NQ = ((nv_max + 63) // 64) * 64   # query width: valid queries only

    if _CACHE.get("NP") != NP or _CACHE.get("NQ") != NQ:
        _CACHE["nc"] = _build_nc(NP, NQ)
        _CACHE["NP"] = NP
        _CACHE["NQ"] = NQ
    nc = _CACHE["nc"]

    in_maps = [
        _prep_core_inputs(inputs, c // HG, c % HG, NP) for c in range(NCORES)
    ]

    res = run_bass_kernel_spmd(nc, in_maps, core_ids=list(range(NCORES)))
    results = res.results

    bp = inputs["bp"].astype(np.float32)
    out = np.empty((B, N, OUT_DIM), np.float32)
    for b in range(B):
        acc = results[b * HG]["outT"].astype(np.float32)
        for g in range(1, HG):
            acc = acc + results[b * HG + g]["outT"].astype(np.float32)
        perm, nv = _perm_np(inputs["mask"][b, :, 0])
        out[b] = bp[None, :]
        out[b, perm[:nv]] = acc.T[:nv] + bp[None, :]
    return out
